# revision 1
# baseline (speedup 1.0000x reference)
"""Trainium2 Bass kernel for a Mixtral decoder layer (attention + top-2 MoE).

Strategy (8 NeuronCores):
  Launch 1 (attention): 2D shard = (batch b in {0,1}) x (head-group g in {0..3},
    4 heads / 256 feature slice each). Each core computes q/k/v projections for
    its slice, transposed-scores flash-style attention (scores computed as
    s^T[tk, tq] so the softmax denominator folds into a ones-column of V), and
    a partial output projection. Host sums the 4 partials per batch.
  Host: residual add, rmsnorm, gating logits, exact top-2 routing, per-expert
    token gather (expert-parallel dispatch done in numpy - free).
  Launch 2 (MoE FFN): expert-parallel - core e owns expert e's w1/w3/w2 and
    processes its routed tokens (padded to capacity C) densely, pipelined over
    512-token blocks.
  Host: scatter-add expert outputs + residual. All matmuls bf16 with fp32 PSUM
    accumulation; softmax/normalization/routing in fp32.
"""
import os
import sys

import numpy as np
import ml_dtypes

for _p in ("/root/.axon_site", "/root/.axon_site/_ro/trn_rl_repo", "/opt/trn_rl_repo"):
    if os.path.isdir(_p) and _p not in sys.path:
        sys.path.append(_p)

import concourse.tile as tile
from concourse import bacc, mybir
from concourse.bass_utils import run_bass_kernel_spmd

BF16 = ml_dtypes.bfloat16
AF = mybir.ActivationFunctionType
ALU = mybir.AluOpType
DT = mybir.dt

H = 1024
S = 2048
B = 2
NH = 16
D = 64
E = 8
I = 2048
T = B * S
EPS = 1e-5

NCORES = 8
NGRP = 4              # head groups (cores per batch)
NHPC = NH // NGRP     # 4 heads per core
DS = NHPC * D         # 256-wide feature slice per core
TQC = 4               # tq chunks of 512
NTK = S // 128        # 16 tk tiles
NCI = H // 128        # 8 contraction chunks

C = 1152              # MoE expert token capacity (per-expert max on this data ~1087)

_CACHE = {}
LAST_RESULTS = []     # BassKernelResults of the last kernel() call (for test harness)
TRACE = os.environ.get("KERNEL_TRACE", "0") == "1"


def _capacity_chunks(cap):
    out, o = [], 0
    while o < cap:
        ln = min(512, cap - o)
        out.append((o, ln))
        o += ln
    return out


def _build_l1():
    nc = bacc.Bacc("TRN2", target_bir_lowering=False, debug=False, num_devices=NCORES)
    xT = nc.dram_tensor("xT", [H, S], DT.bfloat16, kind="ExternalInput")
    wqT = nc.dram_tensor("wqT", [H, DS], DT.bfloat16, kind="ExternalInput")
    wkT = nc.dram_tensor("wkT", [H, DS], DT.bfloat16, kind="ExternalInput")
    wvT = nc.dram_tensor("wvT", [H, DS], DT.bfloat16, kind="ExternalInput")
    woT = nc.dram_tensor("woT", [DS, H], DT.bfloat16, kind="ExternalInput")
    h1p = nc.dram_tensor("h1p", [S, H], DT.float32, kind="ExternalOutput")

    with tile.TileContext(nc) as tc:
        with tc.tile_pool(name="wpool", bufs=1) as wpool, \
             tc.tile_pool(name="qk", bufs=1) as qkpool, \
             tc.tile_pool(name="vp", bufs=1) as vpool, \
             tc.tile_pool(name="pt", bufs=4) as ptpool, \
             tc.tile_pool(name="ao", bufs=1) as aopool, \
             tc.tile_pool(name="rc", bufs=4) as rcpool, \
             tc.tile_pool(name="avs", bufs=3) as avspool, \
             tc.tile_pool(name="hout", bufs=4) as hpool, \
             tc.tile_pool(name="dram", bufs=2, space="DRAM") as drpool, \
             tc.tile_pool(name="pp", bufs=2, space="PSUM") as pp, \
             tc.tile_pool(name="pav", bufs=4, space="PSUM") as pav:

            # ---- load inputs, ordered by first use: early xT chunks + wq/wk
            # first, later xT chunks next, wv/wo last ----
            xts = [wpool.tile([128, S], DT.bfloat16, name=f"xt{ci}", tag=f"xt{ci}")
                   for ci in range(NCI)]
            for ci in range(3):
                nc.sync.dma_start(xts[ci][:], xT.rearrange("(c p) s -> c p s", p=128)[ci])
            wq_sb = wpool.tile([128, NCI, DS], DT.bfloat16)
            nc.sync.dma_start(wq_sb[:], wqT.rearrange("(c p) m -> p c m", p=128))
            wk_sb = wpool.tile([128, NCI, DS], DT.bfloat16)
            nc.sync.dma_start(wk_sb[:], wkT.rearrange("(c p) m -> p c m", p=128))
            for ci in range(3, NCI):
                nc.sync.dma_start(xts[ci][:], xT.rearrange("(c p) s -> c p s", p=128)[ci])
            wv_sb = wpool.tile([128, NCI, DS], DT.bfloat16)
            nc.sync.dma_start(wv_sb[:], wvT.rearrange("(c p) m -> p c m", p=128))
            wo_sb = wpool.tile([128, DS // 128, H], DT.bfloat16)
            nc.sync.dma_start(wo_sb[:], woT.rearrange("(c p) m -> p c m", p=128))

            # per-head-pair qT/kT tiles, per-head v tiles (fine dep granularity)
            qts = [qkpool.tile([64, 2, S], DT.bfloat16, name=f"q{p}", tag=f"q{p}")
                   for p in range(NHPC // 2)]
            kts = [qkpool.tile([64, 2, S], DT.bfloat16, name=f"k{p}", tag=f"k{p}")
                   for p in range(NHPC // 2)]
            vts = [vpool.tile([128, NTK, 66], DT.bfloat16, name=f"v{h}", tag=f"v{h}")
                   for h in range(NHPC)]

            def make_qk(pair, wsb, dst):
                # dst[64, 2, S] for heads (2*pair, 2*pair+1)
                for th in range(2):
                    ps = pp.tile([128, 1024], DT.float32, tag="pp", name="ps")
                    for ci in range(NCI):
                        for i, q in enumerate((2 * th, 2 * th + 1)):
                            nc.tensor.matmul(
                                ps[:, i * 512:(i + 1) * 512],
                                wsb[:, ci, pair * 128:(pair + 1) * 128],
                                xts[ci][:, q * 512:(q + 1) * 512],
                                start=(ci == 0), stop=(ci == NCI - 1),
                            )
                    nc.vector.tensor_copy(
                        dst[0:64, 0, 2 * th * 512:(2 * th + 2) * 512], ps[0:64, :])
                    nc.vector.tensor_copy(
                        dst[0:64, 1, 2 * th * 512:(2 * th + 2) * 512], ps[64:128, :])

            def make_v():
                for h in range(NHPC):
                    nc.vector.memset(vts[h][:, :, 64:66], 0.0)
                    nc.vector.memset(vts[h][:, :, 64:65], 1.0)
                for tkt in range(NTK):
                    pv = pp.tile([128, 1024], DT.float32, tag="pp", name="pv")
                    for ci in range(NCI):
                        nc.tensor.matmul(
                            pv[:, 0:DS],
                            xts[ci][:, tkt * 128:(tkt + 1) * 128],
                            wv_sb[:, ci, 0:DS],
                            start=(ci == 0), stop=(ci == NCI - 1),
                        )
                    for h in range(NHPC):
                        nc.vector.tensor_copy(
                            vts[h][:, tkt, 0:64], pv[:, h * 64:(h + 1) * 64])

            def attend(h, tqh):
                # one tq half (1024 queries) of head h
                qt, kt, vt = qts[h // 2], kts[h // 2], vts[h]
                hi = h % 2
                av = [pav.tile([65, 512], DT.float32, tag="pav", name=f"av{q}")
                      for q in range(2)]

                def av_mms(pt, tkt):
                    for i in range(2):
                        nc.tensor.matmul(
                            av[i][:],
                            vt[:, tkt, 0:65],
                            pt[:, i, :],
                            start=(tkt == 0), stop=(tkt == NTK - 1),
                        )

                # software pipeline: AV for iteration t is emitted after the
                # scores+exp of t+1, so PE never waits on ACT's exp latency.
                pending = None
                for tkt in range(NTK):
                    pt = ptpool.tile([128, 2, 512], DT.bfloat16, tag="pt")
                    sc = pp.tile([128, 1024], DT.float32, tag="pp", name="sc")
                    for i in range(2):
                        q = 2 * tqh + i
                        nc.tensor.matmul(
                            sc[:, i * 512:(i + 1) * 512],
                            kt[0:64, hi, tkt * 128:(tkt + 1) * 128],
                            qt[0:64, hi, q * 512:(q + 1) * 512],
                            start=True, stop=True,
                        )
                    nc.scalar.activation(
                        pt[:],
                        sc[:].rearrange("p (a b) -> p a b", b=512),
                        AF.Exp, scale=0.125)
                    if pending is not None:
                        av_mms(*pending)
                    pending = (pt, tkt)
                av_mms(*pending)
                # evict AV psum to SBUF right away (frees pav slots)
                av_sb = avspool.tile([65, 1024], DT.float32, tag="avs", name="av_sb")
                for i in range(2):
                    nc.vector.tensor_copy(av_sb[:, i * 512:(i + 1) * 512], av[i][:])
                rc = rcpool.tile([1, 1024], DT.float32, tag="rc")
                nc.vector.reciprocal(rc[0:1, :], av_sb[64:65, :])
                rd = drpool.tile([1, 1024], DT.float32)
                nc.sync.dma_start(rd[:], rc[:])
                rb = rcpool.tile([64, 1024], DT.float32, tag="rb")
                nc.sync.dma_start(rb[:], rd[:].to_broadcast([64, 1024]))
                roff = (h % 2) * 64
                for i in range(2):
                    q = 2 * tqh + i
                    nc.vector.tensor_tensor(
                        aoT_sb[roff:roff + 64, h // 2, q * 512:(q + 1) * 512],
                        av_sb[0:64, i * 512:(i + 1) * 512],
                        rb[:, i * 512:(i + 1) * 512],
                        ALU.mult,
                    )

            aoT_sb = aopool.tile([128, DS // 128, S], DT.bfloat16)
            make_qk(0, wq_sb, qts[0])
            make_qk(0, wk_sb, kts[0])
            make_v()
            attend(0, 0)
            make_qk(1, wq_sb, qts[1])   # overlaps ACT-bound attends of pair 0
            make_qk(1, wk_sb, kts[1])
            attend(0, 1)
            attend(1, 0)
            attend(1, 1)
            attend(2, 0)
            attend(2, 1)
            attend(3, 0)
            attend(3, 1)

            # ---- partial O-projection: h1p[t, :] = sum_o aoT[o, t] * woT[o, :] ----
            for tkt in range(NTK):
                ht = hpool.tile([128, H], DT.float32, tag="ht")
                po = pp.tile([128, 1024], DT.float32, tag="pp", name="po")
                for jc in range(H // 512):
                    for oc in range(DS // 128):
                        nc.tensor.matmul(
                            po[:, jc * 512:(jc + 1) * 512],
                            aoT_sb[:, oc, tkt * 128:(tkt + 1) * 128],
                            wo_sb[:, oc, jc * 512:(jc + 1) * 512],
                            start=(oc == 0), stop=(oc == DS // 128 - 1),
                        )
                nc.vector.tensor_copy(ht[:], po[:])
                nc.sync.dma_start(h1p[tkt * 128:(tkt + 1) * 128, :], ht[:])

    nc.compile()
    nc.finalize()
    return nc


def _build_l2(cap):
    nc = bacc.Bacc("TRN2", target_bir_lowering=False, debug=False, num_devices=NCORES)
    zeT = nc.dram_tensor("zeT", [H, cap], DT.bfloat16, kind="ExternalInput")
    w1T = nc.dram_tensor("w1T", [H, I], DT.bfloat16, kind="ExternalInput")
    w3T = nc.dram_tensor("w3T", [H, I], DT.bfloat16, kind="ExternalInput")
    w2T = nc.dram_tensor("w2T", [I, H], DT.bfloat16, kind="ExternalInput")
    web = nc.dram_tensor("web", [128, cap], DT.float32, kind="ExternalInput")
    yT = nc.dram_tensor("yT", [H, cap], DT.float32, kind="ExternalOutput")

    cch = _capacity_chunks(cap)
    NIC = I // 128
    with tile.TileContext(nc) as tc:
        with tc.tile_pool(name="wpool", bufs=1) as wpool, \
             tc.tile_pool(name="hh", bufs=1) as hhpool, \
             tc.tile_pool(name="hs", bufs=3) as hspool, \
             tc.tile_pool(name="yt", bufs=3) as ytpool, \
             tc.tile_pool(name="pg", bufs=6, space="PSUM") as pg, \
             tc.tile_pool(name="py", bufs=2, space="PSUM") as py:

            # per-chunk tiles + first-use-ordered loads so the first h-matmul
            # only waits for chunk 0 of z and w1
            zcs = [wpool.tile([128, cap], DT.bfloat16, name=f"zc{c}", tag=f"zc{c}")
                   for c in range(NCI)]
            w1cs = [wpool.tile([128, I], DT.bfloat16, name=f"w1c{c}", tag=f"w1c{c}")
                    for c in range(NCI)]
            w3cs = [wpool.tile([128, I], DT.bfloat16, name=f"w3c{c}", tag=f"w3c{c}")
                    for c in range(NCI)]
            for c in range(NCI):
                nc.sync.dma_start(zcs[c][:], zeT.rearrange("(c p) m -> c p m", p=128)[c])
                nc.sync.dma_start(w1cs[c][:], w1T.rearrange("(c p) m -> c p m", p=128)[c])
                nc.sync.dma_start(w3cs[c][:], w3T.rearrange("(c p) m -> c p m", p=128)[c])
            web_sb = wpool.tile([128, cap], DT.float32)
            nc.sync.dma_start(web_sb[:], web[:, :])

            # hh in two halves so the y-phase can start after the first half
            hhs = [hhpool.tile([128, NIC // 2, cap], DT.bfloat16, name=f"hh{a}",
                               tag=f"hh{a}") for a in range(2)]
            w2_holder = []

            for ic in range(NIC):
                hp = [pg.tile([128, 512], DT.float32, tag="pg", name=f"hp{j}")
                      for j in range(len(cch))]
                for hc in range(NCI):
                    for j, (o, ln) in enumerate(cch):
                        nc.tensor.matmul(
                            hp[j][:, 0:ln],
                            w1cs[hc][:, ic * 128:(ic + 1) * 128],
                            zcs[hc][:, o:o + ln],
                            start=(hc == 0), stop=(hc == NCI - 1),
                        )
                hs = hspool.tile([128, cap], DT.bfloat16, tag="hs", name="hs")
                for j, (o, ln) in enumerate(cch):
                    nc.scalar.activation(hs[:, o:o + ln], hp[j][:, 0:ln], AF.Silu)
                gp = [pg.tile([128, 512], DT.float32, tag="pg", name=f"gp{j}")
                      for j in range(len(cch))]
                for hc in range(NCI):
                    for j, (o, ln) in enumerate(cch):
                        nc.tensor.matmul(
                            gp[j][:, 0:ln],
                            w3cs[hc][:, ic * 128:(ic + 1) * 128],
                            zcs[hc][:, o:o + ln],
                            start=(hc == 0), stop=(hc == NCI - 1),
                        )
                for j, (o, ln) in enumerate(cch):
                    nc.vector.tensor_tensor(
                        hhs[ic // (NIC // 2)][:, ic % (NIC // 2), o:o + ln],
                        gp[j][:, 0:ln], hs[:, o:o + ln], ALU.mult)
                if ic == 0:
                    # emit w2 load after the first h-block for DMA priority
                    w2_sb = wpool.tile([128, NIC, H], DT.bfloat16)
                    nc.sync.dma_start(
                        w2_sb[:], w2T.rearrange("(c p) m -> p c m", p=128))
                    w2_holder.append(w2_sb)

            w2_sb = w2_holder[0]
            for hc in range(NCI):
                yt = ytpool.tile([128, cap], DT.float32, tag="yt", name="yt")
                for j, (o, ln) in enumerate(cch):
                    yp = py.tile([128, 512], DT.float32, tag="py", name="yp")
                    for ic in range(NIC):
                        nc.tensor.matmul(
                            yp[:, 0:ln],
                            w2_sb[:, ic, hc * 128:(hc + 1) * 128],
                            hhs[ic // (NIC // 2)][:, ic % (NIC // 2), o:o + ln],
                            start=(ic == 0), stop=(ic == NIC - 1),
                        )
                    nc.vector.tensor_tensor(
                        yt[:, o:o + ln], yp[:, 0:ln], web_sb[:, o:o + ln], ALU.mult)
                nc.sync.dma_start(yT[hc * 128:(hc + 1) * 128, :], yt[:])

    nc.compile()
    nc.finalize()
    return nc


def _get(name, builder, *args):
    if name not in _CACHE:
        _CACHE[name] = builder(*args)
    return _CACHE[name]


def _rmsnorm(x, w):
    xf = x.astype(np.float32)
    rms = 1.0 / np.sqrt((xf * xf).mean(axis=-1, keepdims=True) + EPS)
    return (xf * rms) * w.astype(np.float32)


def kernel(x, ln1_w, ln2_w, wq, wk, wv, wo, gate_w, w1, w2, w3):
    global LAST_RESULTS
    LAST_RESULTS = []
    x = np.asarray(x, np.float32)
    wq, wk, wv, wo = (np.asarray(a, np.float32) for a in (wq, wk, wv, wo))
    gate_w = np.asarray(gate_w, np.float32)
    w1, w2, w3 = (np.asarray(a, np.float32) for a in (w1, w2, w3))
    ln1_w = np.asarray(ln1_w, np.float32)
    ln2_w = np.asarray(ln2_w, np.float32)

    xf = x.reshape(T, H)
    z1 = _rmsnorm(xf, ln1_w)
    # ---- launch 1: attention ----
    nc1 = _get("l1", _build_l1)
    in_maps = []
    for c in range(NCORES):
        b, g = divmod(c, NGRP)
        sl = slice(g * DS, (g + 1) * DS)
        in_maps.append({
            "xT": np.ascontiguousarray(z1[b * S:(b + 1) * S].T).astype(BF16),
            "wqT": np.ascontiguousarray(wq[sl].T).astype(BF16),
            "wkT": np.ascontiguousarray(wk[sl].T).astype(BF16),
            "wvT": np.ascontiguousarray(wv[sl].T).astype(BF16),
            "woT": np.ascontiguousarray(wo[:, sl].T).astype(BF16),
        })
    res1 = run_bass_kernel_spmd(nc1, in_maps, core_ids=list(range(NCORES)), trace=TRACE)
    LAST_RESULTS.append(res1)

    h1 = xf.copy()
    for c in range(NCORES):
        b = c // NGRP
        h1[b * S:(b + 1) * S] += res1.results[c]["h1p"]

    # ---- host: routing (exact fp32 semantics like the reference) ----
    z = _rmsnorm(h1, ln2_w)
    logits = (z.astype(np.float64) @ gate_w.T.astype(np.float64)).astype(np.float32)
    order = np.argsort(-logits, axis=-1, kind="stable")
    sel = order[:, :2]                               # top-2, ties -> lower index
    vals = np.take_along_axis(logits, sel, axis=-1).astype(np.float32)
    mx = vals.max(axis=-1, keepdims=True)
    ex = np.exp(vals - mx)
    rw = (ex / ex.sum(axis=-1, keepdims=True)).astype(np.float32)

    idx_lists = []
    for e in range(E):
        m = (sel == e)
        tok = np.nonzero(m.any(axis=-1))[0]
        wgt = np.where(m, rw, 0.0).sum(axis=-1)[tok]
        idx_lists.append((tok, wgt.astype(np.float32)))
    maxload = max(len(tok) for tok, _ in idx_lists)
    cap = C
    while cap < maxload:
        cap += 512
    nc2 = _get(f"l2_{cap}", _build_l2, cap)

    # ---- launch 2: expert-parallel FFN ----
    zT = np.ascontiguousarray(z.T).astype(BF16)      # [H, T]
    in_maps2 = []
    for e in range(E):
        tok, wgt = idx_lists[e]
        zeT = np.zeros((H, cap), BF16)
        zeT[:, :len(tok)] = zT[:, tok]
        web = np.zeros((cap,), np.float32)
        web[:len(tok)] = wgt
        in_maps2.append({
            "zeT": zeT,
            "w1T": np.ascontiguousarray(w1[e].T).astype(BF16),
            "w3T": np.ascontiguousarray(w3[e].T).astype(BF16),
            "w2T": np.ascontiguousarray(w2[e].T).astype(BF16),
            "web": np.broadcast_to(web, (128, cap)).copy(),
        })
    res2 = run_bass_kernel_spmd(nc2, in_maps2, core_ids=list(range(NCORES)), trace=TRACE)
    LAST_RESULTS.append(res2)

    out = h1.copy()
    for e in range(E):
        tok, _ = idx_lists[e]
        out[tok] += res2.results[e]["yT"][:, :len(tok)].T

    return out.reshape(B, S, H).astype(np.float32)



# revision 5
# speedup vs baseline: 1.4546x; 1.4546x over previous
"""Trainium2 Bass kernel for a Mixtral decoder layer (attention + top-2 MoE).

Strategy (8 NeuronCores):
  Launch 1 (attention): 2D shard = (batch b in {0,1}) x (head-group g in {0..3},
    4 heads / 256 feature slice each). Each core computes q/k/v projections for
    its slice, transposed-scores flash-style attention (scores computed as
    s^T[tk, tq] so the softmax denominator folds into a ones-column of V), and
    a partial output projection. Host sums the 4 partials per batch.
  Host: residual add, rmsnorm, gating logits, exact top-2 routing, per-expert
    token gather (expert-parallel dispatch done in numpy - free).
  Launch 2 (MoE FFN): expert-parallel - core e owns expert e's w1/w3/w2 and
    processes its routed tokens (padded to capacity C) densely, pipelined over
    512-token blocks.
  Host: scatter-add expert outputs + residual. All matmuls bf16 with fp32 PSUM
    accumulation; softmax/normalization/routing in fp32.
"""
import os
import sys

import numpy as np
import ml_dtypes

for _p in ("/root/.axon_site", "/root/.axon_site/_ro/trn_rl_repo", "/opt/trn_rl_repo"):
    if os.path.isdir(_p) and _p not in sys.path:
        sys.path.append(_p)

import concourse.tile as tile
from concourse import bacc, mybir
from concourse.bass_utils import run_bass_kernel_spmd

BF16 = ml_dtypes.bfloat16
AF = mybir.ActivationFunctionType
ALU = mybir.AluOpType
DT = mybir.dt

H = 1024
S = 2048
B = 2
NH = 16
D = 64
E = 8
I = 2048
T = B * S
EPS = 1e-5

NCORES = 8
NGRP = 4              # head groups (cores per batch)
NHPC = NH // NGRP     # 4 heads per core
DS = NHPC * D         # 256-wide feature slice per core
TQC = 4               # tq chunks of 512
NTK = S // 128        # 16 tk tiles
NCI = H // 128        # 8 contraction chunks

C = 1088              # MoE expert token capacity (per-expert max on this data ~1087)
SW1, SW3, SW2 = 64.0, 16.0, 64.0   # fp8 weight scales (powers of 2, exact to undo)
F8 = ml_dtypes.float8_e4m3

_CACHE = {}
LAST_RESULTS = []     # BassKernelResults of the last kernel() call (for test harness)
TRACE = os.environ.get("KERNEL_TRACE", "0") == "1"


def _capacity_chunks(cap):
    out, o = [], 0
    while o < cap:
        ln = min(512, cap - o)
        out.append((o, ln))
        o += ln
    return out


def _build_l1():
    nc = bacc.Bacc("TRN2", target_bir_lowering=False, debug=False, num_devices=NCORES)
    xT = nc.dram_tensor("xT", [H, S], DT.bfloat16, kind="ExternalInput")
    wqT = nc.dram_tensor("wqT", [H, DS], DT.bfloat16, kind="ExternalInput")
    wkT = nc.dram_tensor("wkT", [H, DS], DT.bfloat16, kind="ExternalInput")
    wvT = nc.dram_tensor("wvT", [H, DS], DT.bfloat16, kind="ExternalInput")
    woT = nc.dram_tensor("woT", [DS, H], DT.bfloat16, kind="ExternalInput")
    h1p = nc.dram_tensor("h1p", [S, H], DT.float32, kind="ExternalOutput")

    with tile.TileContext(nc) as tc:
        with tc.tile_pool(name="wpool", bufs=1) as wpool, \
             tc.tile_pool(name="qk", bufs=1) as qkpool, \
             tc.tile_pool(name="vp", bufs=1) as vpool, \
             tc.tile_pool(name="pt", bufs=4) as ptpool, \
             tc.tile_pool(name="ao", bufs=1) as aopool, \
             tc.tile_pool(name="rc", bufs=4) as rcpool, \
             tc.tile_pool(name="avs", bufs=3) as avspool, \
             tc.tile_pool(name="hout", bufs=4) as hpool, \
             tc.tile_pool(name="dram", bufs=2, space="DRAM") as drpool, \
             tc.tile_pool(name="pp", bufs=2, space="PSUM") as pp, \
             tc.tile_pool(name="pav", bufs=4, space="PSUM") as pav:

            # ---- load inputs, ordered by first use: early xT chunks + wq/wk
            # first, later xT chunks next, wv/wo last ----
            xts = [wpool.tile([128, S], DT.bfloat16, name=f"xt{ci}", tag=f"xt{ci}")
                   for ci in range(NCI)]
            for ci in range(3):
                nc.sync.dma_start(xts[ci][:], xT.rearrange("(c p) s -> c p s", p=128)[ci])
            wq_sb = wpool.tile([128, NCI, DS], DT.bfloat16)
            nc.sync.dma_start(wq_sb[:], wqT.rearrange("(c p) m -> p c m", p=128))
            wk_sb = wpool.tile([128, NCI, DS], DT.bfloat16)
            nc.sync.dma_start(wk_sb[:], wkT.rearrange("(c p) m -> p c m", p=128))
            for ci in range(3, NCI):
                nc.sync.dma_start(xts[ci][:], xT.rearrange("(c p) s -> c p s", p=128)[ci])
            wv_sb = wpool.tile([128, NCI, DS], DT.bfloat16)
            nc.sync.dma_start(wv_sb[:], wvT.rearrange("(c p) m -> p c m", p=128))
            wo_sb = wpool.tile([128, DS // 128, H], DT.bfloat16)
            nc.sync.dma_start(wo_sb[:], woT.rearrange("(c p) m -> p c m", p=128))

            # per-head-pair qT/kT tiles, per-head v tiles (fine dep granularity)
            qts = [qkpool.tile([64, 2, S], DT.bfloat16, name=f"q{p}", tag=f"q{p}")
                   for p in range(NHPC // 2)]
            kts = [qkpool.tile([64, 2, S], DT.bfloat16, name=f"k{p}", tag=f"k{p}")
                   for p in range(NHPC // 2)]
            vts = [vpool.tile([128, NTK, 66], DT.bfloat16, name=f"v{h}", tag=f"v{h}")
                   for h in range(NHPC)]

            def make_qk(pair, wsb, dst):
                # dst[64, 2, S] for heads (2*pair, 2*pair+1)
                for th in range(2):
                    ps = pp.tile([128, 1024], DT.float32, tag="pp", name="ps")
                    for ci in range(NCI):
                        for i, q in enumerate((2 * th, 2 * th + 1)):
                            nc.tensor.matmul(
                                ps[:, i * 512:(i + 1) * 512],
                                wsb[:, ci, pair * 128:(pair + 1) * 128],
                                xts[ci][:, q * 512:(q + 1) * 512],
                                start=(ci == 0), stop=(ci == NCI - 1),
                            )
                    nc.vector.tensor_copy(
                        dst[0:64, 0, 2 * th * 512:(2 * th + 2) * 512], ps[0:64, :])
                    nc.vector.tensor_copy(
                        dst[0:64, 1, 2 * th * 512:(2 * th + 2) * 512], ps[64:128, :])

            def make_v():
                for h in range(NHPC):
                    nc.vector.memset(vts[h][:, :, 64:66], 0.0)
                    nc.vector.memset(vts[h][:, :, 64:65], 1.0)
                for tkt in range(NTK):
                    pv = pp.tile([128, 1024], DT.float32, tag="pp", name="pv")
                    for ci in range(NCI):
                        nc.tensor.matmul(
                            pv[:, 0:DS],
                            xts[ci][:, tkt * 128:(tkt + 1) * 128],
                            wv_sb[:, ci, 0:DS],
                            start=(ci == 0), stop=(ci == NCI - 1),
                        )
                    for h in range(NHPC):
                        nc.vector.tensor_copy(
                            vts[h][:, tkt, 0:64], pv[:, h * 64:(h + 1) * 64])

            def attend(h, tqh):
                # one tq half (1024 queries) of head h
                qt, kt, vt = qts[h // 2], kts[h // 2], vts[h]
                hi = h % 2
                av = [pav.tile([65, 512], DT.float32, tag="pav", name=f"av{q}")
                      for q in range(2)]

                def av_mms(pt, tkt):
                    for i in range(2):
                        nc.tensor.matmul(
                            av[i][:],
                            vt[:, tkt, 0:65],
                            pt[:, i, :],
                            start=(tkt == 0), stop=(tkt == NTK - 1),
                        )

                # software pipeline: AV for iteration t is emitted after the
                # scores+exp of t+1, so PE never waits on ACT's exp latency.
                pending = None
                for tkt in range(NTK):
                    pt = ptpool.tile([128, 2, 512], DT.bfloat16, tag="pt")
                    sc = pp.tile([128, 1024], DT.float32, tag="pp", name="sc")
                    for i in range(2):
                        q = 2 * tqh + i
                        nc.tensor.matmul(
                            sc[:, i * 512:(i + 1) * 512],
                            kt[0:64, hi, tkt * 128:(tkt + 1) * 128],
                            qt[0:64, hi, q * 512:(q + 1) * 512],
                            start=True, stop=True,
                        )
                    nc.scalar.activation(
                        pt[:],
                        sc[:].rearrange("p (a b) -> p a b", b=512),
                        AF.Exp, scale=0.125)
                    if pending is not None:
                        av_mms(*pending)
                    pending = (pt, tkt)
                av_mms(*pending)
                # evict AV psum to SBUF right away (frees pav slots)
                av_sb = avspool.tile([65, 1024], DT.float32, tag="avs", name="av_sb")
                for i in range(2):
                    nc.vector.tensor_copy(av_sb[:, i * 512:(i + 1) * 512], av[i][:])
                rc = rcpool.tile([1, 1024], DT.float32, tag="rc")
                nc.vector.reciprocal(rc[0:1, :], av_sb[64:65, :])
                rd = drpool.tile([1, 1024], DT.float32)
                nc.sync.dma_start(rd[:], rc[:])
                rb = rcpool.tile([64, 1024], DT.float32, tag="rb")
                nc.sync.dma_start(rb[:], rd[:].to_broadcast([64, 1024]))
                roff = (h % 2) * 64
                for i in range(2):
                    q = 2 * tqh + i
                    nc.vector.tensor_tensor(
                        aoT_sb[roff:roff + 64, h // 2, q * 512:(q + 1) * 512],
                        av_sb[0:64, i * 512:(i + 1) * 512],
                        rb[:, i * 512:(i + 1) * 512],
                        ALU.mult,
                    )

            aoT_sb = aopool.tile([128, DS // 128, S], DT.bfloat16)
            make_qk(0, wq_sb, qts[0])
            make_qk(0, wk_sb, kts[0])
            make_v()
            attend(0, 0)
            make_qk(1, wq_sb, qts[1])   # overlaps ACT-bound attends of pair 0
            make_qk(1, wk_sb, kts[1])
            attend(0, 1)
            attend(1, 0)
            attend(1, 1)
            attend(2, 0)
            attend(2, 1)
            attend(3, 0)
            attend(3, 1)

            # ---- partial O-projection: h1p[t, :] = sum_o aoT[o, t] * woT[o, :] ----
            for tkt in range(NTK):
                ht = hpool.tile([128, H], DT.float32, tag="ht")
                po = pp.tile([128, 1024], DT.float32, tag="pp", name="po")
                for jc in range(H // 512):
                    for oc in range(DS // 128):
                        nc.tensor.matmul(
                            po[:, jc * 512:(jc + 1) * 512],
                            aoT_sb[:, oc, tkt * 128:(tkt + 1) * 128],
                            wo_sb[:, oc, jc * 512:(jc + 1) * 512],
                            start=(oc == 0), stop=(oc == DS // 128 - 1),
                        )
                nc.vector.tensor_copy(ht[:], po[:])
                nc.sync.dma_start(h1p[tkt * 128:(tkt + 1) * 128, :], ht[:])

    nc.compile()
    nc.finalize()
    return nc


def _build_l2(cap):
    """Expert-parallel MoE FFN, fp8e4m3 + DoubleRow (2 K-tiles/instruction).

    Scales: w1 *= SW1 (silu scale=1/SW1 undoes exactly), w3 *= SW3,
    w2 *= SW2; web divided by SW3*SW2 on host. hh = silu_true * SW3*g_true
    stays well under fp8e4 max 240 for SW3=16."""
    nc = bacc.Bacc("TRN2", target_bir_lowering=False, debug=False, num_devices=NCORES)
    zeT = nc.dram_tensor("zeT", [H, cap], DT.float8e4, kind="ExternalInput")
    w1T = nc.dram_tensor("w1T", [H, I], DT.float8e4, kind="ExternalInput")
    w3T = nc.dram_tensor("w3T", [H, I], DT.float8e4, kind="ExternalInput")
    w2T = nc.dram_tensor("w2T", [I, H], DT.float8e4, kind="ExternalInput")
    web = nc.dram_tensor("web", [128, cap], DT.float32, kind="ExternalInput")
    yT = nc.dram_tensor("yT", [H, cap], DT.bfloat16, kind="ExternalOutput")

    cch = _capacity_chunks(cap)
    NIC = I // 128
    NPH = H // 256        # 4 K-pairs over H
    NPI = I // 256        # 8 K-pairs over I
    DR = mybir.MatmulPerfMode.DoubleRow
    with tile.TileContext(nc) as tc:
        with tc.tile_pool(name="wpool", bufs=1) as wpool, \
             tc.tile_pool(name="hh", bufs=1) as hhpool, \
             tc.tile_pool(name="hs", bufs=3) as hspool, \
             tc.tile_pool(name="yt", bufs=3) as ytpool, \
             tc.tile_pool(name="pg", bufs=6, space="PSUM") as pg, \
             tc.tile_pool(name="py", bufs=2, space="PSUM") as py:

            # K-pair tiles, first-use-ordered loads: the h matmul for pair p
            # only waits for z pair p + w1 pair p
            zps = [wpool.tile([128, 2, cap], DT.float8e4, name=f"zp{p}", tag=f"zp{p}")
                   for p in range(NPH)]
            w1ps = [wpool.tile([128, 2, I], DT.float8e4, name=f"w1p{p}", tag=f"w1p{p}")
                    for p in range(NPH)]
            w3ps = [wpool.tile([128, 2, I], DT.float8e4, name=f"w3p{p}", tag=f"w3p{p}")
                    for p in range(NPH)]
            zr = zeT.rearrange("(a two p) m -> a p two m", p=128, two=2)
            w1r = w1T.rearrange("(a two p) m -> a p two m", p=128, two=2)
            w3r = w3T.rearrange("(a two p) m -> a p two m", p=128, two=2)
            for p in range(NPH):
                nc.sync.dma_start(zps[p][:], zr[p])
                nc.sync.dma_start(w1ps[p][:], w1r[p])
            for p in range(NPH):
                nc.sync.dma_start(w3ps[p][:], w3r[p])
            web_sb = wpool.tile([128, cap], DT.float32)
            nc.sync.dma_start(web_sb[:], web[:, :])

            # hh as K-pair tiles over I for the DoubleRow y-phase
            hhp = [hhpool.tile([128, 2, cap], DT.float8e4, name=f"hhp{p}",
                               tag=f"hhp{p}") for p in range(NPI)]
            w2_holder = []

            for ic in range(NIC):
                hp = [pg.tile([128, 512], DT.float32, tag="pg", name=f"hp{j}")
                      for j in range(len(cch))]
                for p in range(NPH):
                    for j, (o, ln) in enumerate(cch):
                        nc.tensor.matmul(
                            hp[j][:, 0:ln],
                            w1ps[p][:, :, ic * 128:(ic + 1) * 128],
                            zps[p][:, :, o:o + ln],
                            start=(p == 0), stop=(p == NPH - 1),
                            perf_mode=DR,
                        )
                hs = hspool.tile([128, cap], DT.float8e4, tag="hs", name="hs")
                for j, (o, ln) in enumerate(cch):
                    nc.scalar.activation(hs[:, o:o + ln], hp[j][:, 0:ln], AF.Silu,
                                         scale=1.0 / SW1)
                gp = [pg.tile([128, 512], DT.float32, tag="pg", name=f"gp{j}")
                      for j in range(len(cch))]
                for p in range(NPH):
                    for j, (o, ln) in enumerate(cch):
                        nc.tensor.matmul(
                            gp[j][:, 0:ln],
                            w3ps[p][:, :, ic * 128:(ic + 1) * 128],
                            zps[p][:, :, o:o + ln],
                            start=(p == 0), stop=(p == NPH - 1),
                            perf_mode=DR,
                        )
                for j, (o, ln) in enumerate(cch):
                    nc.vector.tensor_tensor(
                        hhp[ic // 2][:, ic % 2, o:o + ln],
                        gp[j][:, 0:ln], hs[:, o:o + ln], ALU.mult)
                if ic == 0:
                    # emit w2 load after the first h-block for DMA priority
                    w2ps = [wpool.tile([128, 2, H], DT.float8e4, name=f"w2p{p}",
                                       tag=f"w2p{p}") for p in range(NPI)]
                    w2r = w2T.rearrange("(a two p) m -> a p two m", p=128, two=2)
                    for p in range(NPI):
                        nc.sync.dma_start(w2ps[p][:], w2r[p])
                    w2_holder.append(w2ps)

            w2ps = w2_holder[0]
            for hc in range(NCI):
                yt = ytpool.tile([128, cap], DT.bfloat16, tag="yt", name="yt")
                for j, (o, ln) in enumerate(cch):
                    yp = py.tile([128, 512], DT.float32, tag="py", name="yp")
                    for p in range(NPI):
                        nc.tensor.matmul(
                            yp[:, 0:ln],
                            w2ps[p][:, :, hc * 128:(hc + 1) * 128],
                            hhp[p][:, :, o:o + ln],
                            start=(p == 0), stop=(p == NPI - 1),
                            perf_mode=DR,
                        )
                    nc.vector.tensor_tensor(
                        yt[:, o:o + ln], yp[:, 0:ln], web_sb[:, o:o + ln], ALU.mult)
                nc.sync.dma_start(yT[hc * 128:(hc + 1) * 128, :], yt[:])

    nc.compile()
    nc.finalize()
    return nc


def _get(name, builder, *args):
    if name not in _CACHE:
        _CACHE[name] = builder(*args)
    return _CACHE[name]


def _rmsnorm(x, w):
    xf = x.astype(np.float32)
    rms = 1.0 / np.sqrt((xf * xf).mean(axis=-1, keepdims=True) + EPS)
    return (xf * rms) * w.astype(np.float32)


def kernel(x, ln1_w, ln2_w, wq, wk, wv, wo, gate_w, w1, w2, w3):
    global LAST_RESULTS
    LAST_RESULTS = []
    x = np.asarray(x, np.float32)
    wq, wk, wv, wo = (np.asarray(a, np.float32) for a in (wq, wk, wv, wo))
    gate_w = np.asarray(gate_w, np.float32)
    w1, w2, w3 = (np.asarray(a, np.float32) for a in (w1, w2, w3))
    ln1_w = np.asarray(ln1_w, np.float32)
    ln2_w = np.asarray(ln2_w, np.float32)

    xf = x.reshape(T, H)
    z1 = _rmsnorm(xf, ln1_w)
    # ---- launch 1: attention ----
    nc1 = _get("l1", _build_l1)
    in_maps = []
    for c in range(NCORES):
        b, g = divmod(c, NGRP)
        sl = slice(g * DS, (g + 1) * DS)
        in_maps.append({
            "xT": np.ascontiguousarray(z1[b * S:(b + 1) * S].T).astype(BF16),
            "wqT": np.ascontiguousarray(wq[sl].T).astype(BF16),
            "wkT": np.ascontiguousarray(wk[sl].T).astype(BF16),
            "wvT": np.ascontiguousarray(wv[sl].T).astype(BF16),
            "woT": np.ascontiguousarray(wo[:, sl].T).astype(BF16),
        })
    res1 = run_bass_kernel_spmd(nc1, in_maps, core_ids=list(range(NCORES)), trace=TRACE)
    LAST_RESULTS.append(res1)

    h1 = xf.copy()
    for c in range(NCORES):
        b = c // NGRP
        h1[b * S:(b + 1) * S] += res1.results[c]["h1p"]

    # ---- host: routing (exact fp32 semantics like the reference) ----
    z = _rmsnorm(h1, ln2_w)
    logits = (z.astype(np.float64) @ gate_w.T.astype(np.float64)).astype(np.float32)
    order = np.argsort(-logits, axis=-1, kind="stable")
    sel = order[:, :2]                               # top-2, ties -> lower index
    vals = np.take_along_axis(logits, sel, axis=-1).astype(np.float32)
    mx = vals.max(axis=-1, keepdims=True)
    ex = np.exp(vals - mx)
    rw = (ex / ex.sum(axis=-1, keepdims=True)).astype(np.float32)

    idx_lists = []
    for e in range(E):
        m = (sel == e)
        tok = np.nonzero(m.any(axis=-1))[0]
        wgt = np.where(m, rw, 0.0).sum(axis=-1)[tok]
        idx_lists.append((tok, wgt.astype(np.float32)))
    maxload = max(len(tok) for tok, _ in idx_lists)
    cap = C
    while cap < maxload:
        cap += 512
    nc2 = _get(f"l2_{cap}", _build_l2, cap)

    # ---- launch 2: expert-parallel FFN (fp8) ----
    zT = np.clip(np.ascontiguousarray(z.T), -240, 240).astype(F8)    # [H, T]
    in_maps2 = []
    for e in range(E):
        tok, wgt = idx_lists[e]
        zeT = np.zeros((H, cap), F8)
        zeT[:, :len(tok)] = zT[:, tok]
        web = np.zeros((cap,), np.float32)
        web[:len(tok)] = wgt / (SW3 * SW2)
        in_maps2.append({
            "zeT": zeT,
            "w1T": np.clip(np.ascontiguousarray(w1[e].T) * SW1, -240, 240).astype(F8),
            "w3T": np.clip(np.ascontiguousarray(w3[e].T) * SW3, -240, 240).astype(F8),
            "w2T": np.clip(np.ascontiguousarray(w2[e].T) * SW2, -240, 240).astype(F8),
            "web": np.broadcast_to(web, (128, cap)).copy(),
        })
    res2 = run_bass_kernel_spmd(nc2, in_maps2, core_ids=list(range(NCORES)), trace=TRACE)
    LAST_RESULTS.append(res2)

    out = h1.copy()
    for e in range(E):
        tok, _ = idx_lists[e]
        out[tok] += res2.results[e]["yT"][:, :len(tok)].T.astype(np.float32)

    return out.reshape(B, S, H).astype(np.float32)



# revision 18
# speedup vs baseline: 1.7363x; 1.1936x over previous
"""Trainium2 Bass kernel for a Mixtral decoder layer (attention + top-2 MoE).

Strategy (8 NeuronCores):
  Launch 1 (attention): 2D shard = (batch b in {0,1}) x (head-group g in {0..3},
    4 heads / 256 feature slice each). Each core computes q/k/v projections for
    its slice, transposed-scores flash-style attention (scores computed as
    s^T[tk, tq] so the softmax denominator folds into a ones-column of V), and
    a partial output projection. Host sums the 4 partials per batch.
  Host: residual add, rmsnorm, gating logits, exact top-2 routing, per-expert
    token gather (expert-parallel dispatch done in numpy - free).
  Launch 2 (MoE FFN): expert-parallel - core e owns expert e's w1/w3/w2 and
    processes its routed tokens (padded to capacity C) densely, pipelined over
    512-token blocks.
  Host: scatter-add expert outputs + residual. All matmuls bf16 with fp32 PSUM
    accumulation; softmax/normalization/routing in fp32.
"""
import os
import sys

import numpy as np
import ml_dtypes

for _p in ("/root/.axon_site", "/root/.axon_site/_ro/trn_rl_repo", "/opt/trn_rl_repo"):
    if os.path.isdir(_p) and _p not in sys.path:
        sys.path.append(_p)

import concourse.tile as tile
from concourse import bacc, mybir
from concourse.bass_utils import run_bass_kernel_spmd

BF16 = ml_dtypes.bfloat16
AF = mybir.ActivationFunctionType
ALU = mybir.AluOpType
DT = mybir.dt

H = 1024
S = 2048
B = 2
NH = 16
D = 64
E = 8
I = 2048
T = B * S
EPS = 1e-5

NCORES = 8
NGRP = 4              # head groups (cores per batch)
NHPC = NH // NGRP     # 4 heads per core
DS = NHPC * D         # 256-wide feature slice per core
TQC = 4               # tq chunks of 512
NTK = S // 128        # 16 tk tiles
NCI = H // 128        # 8 contraction chunks

C = 1088              # MoE expert token capacity (per-expert max on this data ~1087)
SW1, SW3, SW2 = 64.0, 16.0, 64.0   # fp8 weight scales (powers of 2, exact to undo)
F8 = ml_dtypes.float8_e4m3

_CACHE = {}
LAST_RESULTS = []     # BassKernelResults of the last kernel() call (for test harness)
TRACE = os.environ.get("KERNEL_TRACE", "0") == "1"


def _capacity_chunks(cap):
    out, o = [], 0
    while o < cap:
        ln = min(512, cap - o)
        out.append((o, ln))
        o += ln
    return out


def _build_l1():
    """Attention, fp8-DoubleRow projections + flipped AV.

    Weights wq/wk/wv scaled by SQK=32 on host (fp8 range); q',k' = 32*true so
    scores = 1024*true, folded into the exp scale 2^-13. v' = 32*true; the
    AV output is 32*attn, normalized by the softmax denom (ones-column of v,
    unscaled), and 1/32 is folded into woT on host. AV is computed transposed:
    out[q_slice(128), d+1(65)] = pt[tk,q].T @ v[tk,65] so the denominator is a
    per-partition scalar and M=128 (full PE array)."""
    nc = bacc.Bacc("TRN2", target_bir_lowering=False, debug=False, num_devices=NCORES)
    xT8 = nc.dram_tensor("xT8", [H, S], DT.float8e4, kind="ExternalInput")
    wq8 = nc.dram_tensor("wq8", [H, DS], DT.float8e4, kind="ExternalInput")
    wk8 = nc.dram_tensor("wk8", [H, DS], DT.float8e4, kind="ExternalInput")
    wv8 = nc.dram_tensor("wv8", [H, DS], DT.float8e4, kind="ExternalInput")
    woT = nc.dram_tensor("woT", [DS, H], DT.bfloat16, kind="ExternalInput")
    h1p = nc.dram_tensor("h1p", [S, H], DT.bfloat16, kind="ExternalOutput")

    NPH = H // 256       # 4 H k-pairs for DoubleRow
    DR = mybir.MatmulPerfMode.DoubleRow
    EXPSC = 0.125 / (32.0 * 32.0)    # softmax 1/8 plus q,k weight scales
    with tile.TileContext(nc) as tc:
        with tc.tile_pool(name="wpool", bufs=1) as wpool, \
             tc.tile_pool(name="qk", bufs=1) as qkpool, \
             tc.tile_pool(name="vp", bufs=1) as vpool, \
             tc.tile_pool(name="pt", bufs=3) as ptpool, \
             tc.tile_pool(name="ao", bufs=1) as aopool, \
             tc.tile_pool(name="at", bufs=3) as atpool, \
             tc.tile_pool(name="rc", bufs=4) as rcpool, \
             tc.tile_pool(name="hout", bufs=4) as hpool, \
             tc.tile_pool(name="pp", bufs=2, space="PSUM") as pp, \
             tc.tile_pool(name="pav", bufs=2, space="PSUM") as pav:

            # ---- loads: x8/wq pairs first, wk next, wv later, wo last ----
            x8 = [[wpool.tile([128, 2, S // 2], DT.float8e4, name=f"x8_{p}_{hf}",
                              tag=f"x8_{p}_{hf}") for hf in range(2)]
                  for p in range(NPH)]
            # q/k weight tiles padded to DS+16 cols: a [.., 2, 128] slice of an
            # exactly-DS tile is fully contiguous, which walrus's LDW
            # optimization claims and then rejects for DoubleRow
            wq_t = [wpool.tile([128, 2, DS + 16], DT.float8e4, name=f"wqt{p}",
                               tag=f"wqt{p}") for p in range(NPH)]
            wk_t = [wpool.tile([128, 2, DS + 16], DT.float8e4, name=f"wkt{p}",
                               tag=f"wkt{p}") for p in range(NPH)]
            wv_t = [wpool.tile([128, 2, DS], DT.float8e4, name=f"wvt{p}",
                               tag=f"wvt{p}") for p in range(NPH)]
            xr = xT8.rearrange("(a two p) s -> a p two s", p=128, two=2)
            wqr = wq8.rearrange("(a two p) m -> a p two m", p=128, two=2)
            wkr = wk8.rearrange("(a two p) m -> a p two m", p=128, two=2)
            wvr = wv8.rearrange("(a two p) m -> a p two m", p=128, two=2)
            for p in range(NPH):
                nc.sync.dma_start(x8[p][0][:], xr[p][:, :, 0:S // 2])
                nc.sync.dma_start(wq_t[p][:, :, 0:DS], wqr[p])
            for p in range(NPH):
                nc.sync.dma_start(wk_t[p][:, :, 0:DS], wkr[p])
                nc.sync.dma_start(x8[p][1][:], xr[p][:, :, S // 2:S])
            for p in range(NPH):
                nc.sync.dma_start(wv_t[p][:], wvr[p])
            wo_sb = wpool.tile([128, DS // 128, H], DT.bfloat16)
            nc.sync.dma_start(wo_sb[:], woT.rearrange("(c p) m -> p c m", p=128))

            # q/k per head-pair [128, S] bf16 (partitions 0:64 = even head's d,
            # 64:128 = odd head's; scaled by 32); v for all heads in one
            # [128, tk-pair, 2, head, 72] fp8 tile (col 64 = ones)
            qts = [qkpool.tile([128, S], DT.bfloat16, name=f"q{p}", tag=f"q{p}")
                   for p in range(NHPC // 2)]
            kts = [qkpool.tile([128, S], DT.bfloat16, name=f"k{p}", tag=f"k{p}")
                   for p in range(NHPC // 2)]
            vall = vpool.tile([128, NTK // 2, 2, NHPC, 72], DT.float8e4)
            nc.vector.memset(vall[:, :, :, :, 64:65], 1.0)
            aoTs = [aopool.tile([128, DS // 128, S // 2], DT.bfloat16,
                                name=f"aoT{hf}", tag=f"aoT{hf}") for hf in range(2)]

            def make_qk(pair, wt, dst):
                # dst[64, 2, S] bf16 for heads (2*pair, 2*pair+1), values 32x
                for th in range(2):
                    ps = pp.tile([128, 1024], DT.float32, tag="pp", name="ps")
                    for i in range(2):
                        for p in range(NPH):
                            nc.tensor.matmul(
                                ps[:, i * 512:(i + 1) * 512],
                                wt[p][:, :, pair * 128:(pair + 1) * 128],
                                x8[p][th][:, :, i * 512:(i + 1) * 512],
                                start=(p == 0), stop=(p == NPH - 1),
                                perf_mode=DR,
                            )
                    nc.vector.tensor_copy(
                        dst[:, th * 1024:(th + 1) * 1024], ps[:, :])

            def make_v():
                for tkt in range(NTK):
                    pv = pp.tile([128, 1024], DT.float32, tag="pp", name="pv")
                    for p in range(NPH):
                        nc.tensor.matmul(
                            pv[:, 0:DS],
                            x8[p][tkt // 8][:, :, (tkt % 8) * 128:(tkt % 8 + 1) * 128],
                            wv_t[p][:, :, :],
                            start=(p == 0), stop=(p == NPH - 1),
                            perf_mode=DR,
                        )
                    nc.vector.tensor_copy(
                        vall[:, tkt // 2, tkt % 2, :, 0:64],
                        pv[:, 0:DS].rearrange("p (h d) -> p h d", d=64))

            def av_mms(h, av, ptp, j):
                for qs in range(8):
                    nc.tensor.matmul(
                        av[:, qs, 0:65],
                        ptp[:, :, qs * 128:(qs + 1) * 128],
                        vall[:, j, :, h, 0:65],
                        start=(j == 0), stop=(j == NTK // 2 - 1),
                        perf_mode=DR,
                    )

            def attend(h, half):
                # one tq half (1024 queries, 8 slices of 128) of head h;
                # av[q_slice, qs, 0:64] = unnormalized attn (32x), [.., 64] = denom
                qt, kt = qts[h // 2], kts[h // 2]
                ro = (h % 2) * 64
                av = pav.tile([128, 8, 128], DT.float32, tag="pav", name="av")
                pending = None
                ptp = None
                for tkt in range(NTK):
                    sc = pp.tile([128, 1024], DT.float32, tag="pp", name="sc")
                    for i in range(2):
                        q0 = half * 1024 + i * 512
                        nc.tensor.matmul(
                            sc[:, i * 512:(i + 1) * 512],
                            kt[ro:ro + 64, tkt * 128:(tkt + 1) * 128],
                            qt[ro:ro + 64, q0:q0 + 512],
                            start=True, stop=True,
                        )
                    if tkt % 2 == 0:
                        ptp = ptpool.tile([128, 2, 1024], DT.float8e4, tag="pt")
                    nc.scalar.activation(ptp[:, tkt % 2, :], sc[:], AF.Exp,
                                         scale=EXPSC)
                    if tkt % 2 == 1:
                        if pending is not None:
                            av_mms(h, av, *pending)
                        pending = (ptp, tkt // 2)
                av_mms(h, av, *pending)
                rec = rcpool.tile([128, 8], DT.float32, tag="rc")
                nc.vector.reciprocal(rec[:, :], av[:, :, 64])
                # two heads of a pair share an at2 tile: cols (h%2)*64..+64;
                # transposed into aoTs after the odd head (see tp_pair)
                if h % 2 == 0:
                    at2[(h // 2, half)] = atpool.tile([128, 8, 128], DT.bfloat16,
                                                      tag="at", name="at2")
                at = at2[(h // 2, half)]
                roff = (h % 2) * 64
                for qs in range(8):
                    nc.vector.tensor_scalar_mul(
                        at[:, qs, roff:roff + 64], av[:, qs, 0:64],
                        rec[:, qs:qs + 1])

            def tp_pair(pair, half):
                at = at2.pop((pair, half))
                for qs in range(8):
                    nc.sync.dma_start_transpose(
                        aoTs[half][:, pair, qs * 128:(qs + 1) * 128],
                        at[:, qs, :])

            def oproj(half):
                # h1p[tq, :] partial for 8 tq tiles of this half (bf16 out)
                for t in range(8):
                    tkt = half * 8 + t
                    po = pav.tile([128, 8, 128], DT.float32, tag="pav", name="po")
                    pof = po.rearrange("p a b -> p (a b)")
                    for jc in range(2):
                        for oc in range(2):
                            nc.tensor.matmul(
                                pof[:, jc * 512:(jc + 1) * 512],
                                aoTs[half][:, oc, t * 128:(t + 1) * 128],
                                wo_sb[:, oc, jc * 512:(jc + 1) * 512],
                                start=(oc == 0), stop=(oc == 1),
                            )
                    ht = hpool.tile([128, H], DT.bfloat16, tag="ht")
                    nc.vector.tensor_copy(ht[:], pof[:])
                    nc.sync.dma_start(h1p[tkt * 128:(tkt + 1) * 128, :], ht[:])

            at2 = {}
            make_qk(0, wq_t, qts[0])
            make_qk(0, wk_t, kts[0])
            make_v()
            attend(0, 0)
            make_qk(1, wq_t, qts[1])   # overlaps ACT-bound attends of pair 0
            make_qk(1, wk_t, kts[1])
            attend(1, 0)
            tp_pair(0, 0)
            attend(2, 0)
            attend(3, 0)
            tp_pair(1, 0)
            oproj(0)
            attend(0, 1)
            attend(1, 1)
            tp_pair(0, 1)
            attend(2, 1)
            attend(3, 1)
            tp_pair(1, 1)
            oproj(1)

    nc.compile()
    nc.finalize()
    return nc


def _build_l2(cap):
    """Expert-parallel MoE FFN, fp8e4m3 + DoubleRow (2 K-tiles/instruction).

    Scales: w1 *= SW1 (silu scale=1/SW1 undoes exactly), w3 *= SW3,
    w2 *= SW2; web divided by SW3*SW2 on host. hh = silu_true * SW3*g_true
    stays well under fp8e4 max 240 for SW3=16."""
    nc = bacc.Bacc("TRN2", target_bir_lowering=False, debug=False, num_devices=NCORES)
    zeT = nc.dram_tensor("zeT", [H, cap], DT.float8e4, kind="ExternalInput")
    w1T = nc.dram_tensor("w1T", [H, I], DT.float8e4, kind="ExternalInput")
    w3T = nc.dram_tensor("w3T", [H, I], DT.float8e4, kind="ExternalInput")
    w2T = nc.dram_tensor("w2T", [I, H], DT.float8e4, kind="ExternalInput")
    web = nc.dram_tensor("web", [128, cap], DT.float32, kind="ExternalInput")
    yT = nc.dram_tensor("yT", [H, cap], DT.bfloat16, kind="ExternalOutput")

    cch = _capacity_chunks(cap)
    NIC = I // 128
    NPH = H // 256        # 4 K-pairs over H
    NPI = I // 256        # 8 K-pairs over I
    DR = mybir.MatmulPerfMode.DoubleRow
    with tile.TileContext(nc) as tc:
        with tc.tile_pool(name="wpool", bufs=1) as wpool, \
             tc.tile_pool(name="hh", bufs=1) as hhpool, \
             tc.tile_pool(name="hs", bufs=3) as hspool, \
             tc.tile_pool(name="yt", bufs=3) as ytpool, \
             tc.tile_pool(name="pg", bufs=6, space="PSUM") as pg, \
             tc.tile_pool(name="py", bufs=2, space="PSUM") as py:

            # K-pair tiles; z split per cap-chunk and w1 per 4-ic block so the
            # first accumulation group starts after ~2 small DMAs
            zps = [[wpool.tile([128, 2, ln], DT.float8e4, name=f"zp{p}_{j}",
                               tag=f"zp{p}_{j}") for j, (o, ln) in enumerate(cch)]
                   for p in range(NPH)]
            w1ps = [[wpool.tile([128, 2, 512], DT.float8e4, name=f"w1p{p}_{b}",
                                tag=f"w1p{p}_{b}") for b in range(4)]
                    for p in range(NPH)]
            w3ps = [[wpool.tile([128, 2, 512], DT.float8e4, name=f"w3p{p}_{b}",
                                tag=f"w3p{p}_{b}") for b in range(4)]
                    for p in range(NPH)]
            zr = zeT.rearrange("(a two p) m -> a p two m", p=128, two=2)
            w1r = w1T.rearrange("(a two p) m -> a p two m", p=128, two=2)
            w3r = w3T.rearrange("(a two p) m -> a p two m", p=128, two=2)
            for j, (o, ln) in enumerate(cch):
                for p in range(NPH):
                    nc.sync.dma_start(zps[p][j][:], zr[p][:, :, o:o + ln])
                    if j == 0:
                        nc.sync.dma_start(w1ps[p][0][:], w1r[p][:, :, 0:512])
            for b in range(1, 4):
                for p in range(NPH):
                    nc.sync.dma_start(w1ps[p][b][:], w1r[p][:, :, b * 512:(b + 1) * 512])
            for b in range(4):
                for p in range(NPH):
                    nc.sync.dma_start(w3ps[p][b][:], w3r[p][:, :, b * 512:(b + 1) * 512])
            web_sb = wpool.tile([128, cap], DT.float32)
            nc.sync.dma_start(web_sb[:], web[:, :])

            # hh as K-pair tiles over I for the DoubleRow y-phase
            hhp = [hhpool.tile([128, 2, cap], DT.float8e4, name=f"hhp{p}",
                               tag=f"hhp{p}") for p in range(NPI)]
            w2_holder = []

            for ic in range(NIC):
                hp = [pg.tile([128, 512], DT.float32, tag="pg", name=f"hp{j}")
                      for j in range(len(cch))]
                for p in range(NPH):
                    for j, (o, ln) in enumerate(cch):
                        nc.tensor.matmul(
                            hp[j][:, 0:ln],
                            w1ps[p][:, :, ic * 128:(ic + 1) * 128],
                            zps[p][:, :, o:o + ln],
                            start=(p == 0), stop=(p == NPH - 1),
                            perf_mode=DR,
                        )
                hs = hspool.tile([128, cap], DT.float8e4, tag="hs", name="hs")
                for j, (o, ln) in enumerate(cch):
                    nc.scalar.activation(hs[:, o:o + ln], hp[j][:, 0:ln], AF.Silu,
                                         scale=1.0 / SW1)
                gp = [pg.tile([128, 512], DT.float32, tag="pg", name=f"gp{j}")
                      for j in range(len(cch))]
                for p in range(NPH):
                    for j, (o, ln) in enumerate(cch):
                        nc.tensor.matmul(
                            gp[j][:, 0:ln],
                            w3ps[p][:, :, ic * 128:(ic + 1) * 128],
                            zps[p][:, :, o:o + ln],
                            start=(p == 0), stop=(p == NPH - 1),
                            perf_mode=DR,
                        )
                for j, (o, ln) in enumerate(cch):
                    nc.vector.tensor_tensor(
                        hhp[ic // 2][:, ic % 2, o:o + ln],
                        gp[j][:, 0:ln], hs[:, o:o + ln], ALU.mult)
                if ic == 0:
                    # emit w2 load after the first h-block for DMA priority
                    w2ps = [wpool.tile([128, 2, H], DT.float8e4, name=f"w2p{p}",
                                       tag=f"w2p{p}") for p in range(NPI)]
                    w2r = w2T.rearrange("(a two p) m -> a p two m", p=128, two=2)
                    for p in range(NPI):
                        nc.sync.dma_start(w2ps[p][:], w2r[p])
                    w2_holder.append(w2ps)

            w2ps = w2_holder[0]
            for hc in range(NCI):
                yt = ytpool.tile([128, cap], DT.bfloat16, tag="yt", name="yt")
                for j, (o, ln) in enumerate(cch):
                    yp = py.tile([128, 512], DT.float32, tag="py", name="yp")
                    for p in range(NPI):
                        nc.tensor.matmul(
                            yp[:, 0:ln],
                            w2ps[p][:, :, hc * 128:(hc + 1) * 128],
                            hhp[p][:, :, o:o + ln],
                            start=(p == 0), stop=(p == NPI - 1),
                            perf_mode=DR,
                        )
                    nc.vector.tensor_tensor(
                        yt[:, o:o + ln], yp[:, 0:ln], web_sb[:, o:o + ln], ALU.mult)
                nc.sync.dma_start(yT[hc * 128:(hc + 1) * 128, :], yt[:])

    nc.compile()
    nc.finalize()
    return nc


def _get(name, builder, *args):
    if name not in _CACHE:
        _CACHE[name] = builder(*args)
    return _CACHE[name]


def _rmsnorm(x, w):
    xf = x.astype(np.float32)
    rms = 1.0 / np.sqrt((xf * xf).mean(axis=-1, keepdims=True) + EPS)
    return (xf * rms) * w.astype(np.float32)


def kernel(x, ln1_w, ln2_w, wq, wk, wv, wo, gate_w, w1, w2, w3):
    global LAST_RESULTS
    LAST_RESULTS = []
    x = np.asarray(x, np.float32)
    wq, wk, wv, wo = (np.asarray(a, np.float32) for a in (wq, wk, wv, wo))
    gate_w = np.asarray(gate_w, np.float32)
    w1, w2, w3 = (np.asarray(a, np.float32) for a in (w1, w2, w3))
    ln1_w = np.asarray(ln1_w, np.float32)
    ln2_w = np.asarray(ln2_w, np.float32)

    xf = x.reshape(T, H)
    z1 = _rmsnorm(xf, ln1_w)
    # ---- launch 1: attention (fp8 projections, bf16 scores/O-proj) ----
    SQK = 32.0
    nc1 = _get("l1", _build_l1)
    z1_8 = np.clip(z1, -240, 240).astype(F8)
    in_maps = []
    for c in range(NCORES):
        b, g = divmod(c, NGRP)
        sl = slice(g * DS, (g + 1) * DS)
        in_maps.append({
            "xT8": np.ascontiguousarray(z1_8[b * S:(b + 1) * S].T),
            "wq8": np.clip(np.ascontiguousarray(wq[sl].T) * SQK, -240, 240).astype(F8),
            "wk8": np.clip(np.ascontiguousarray(wk[sl].T) * SQK, -240, 240).astype(F8),
            "wv8": np.clip(np.ascontiguousarray(wv[sl].T) * SQK, -240, 240).astype(F8),
            "woT": (np.ascontiguousarray(wo[:, sl].T) / SQK).astype(BF16),
        })
    res1 = run_bass_kernel_spmd(nc1, in_maps, core_ids=list(range(NCORES)), trace=TRACE)
    LAST_RESULTS.append(res1)

    h1 = xf.copy()
    for c in range(NCORES):
        b = c // NGRP
        h1[b * S:(b + 1) * S] += res1.results[c]["h1p"].astype(np.float32)

    # ---- host: routing (exact fp32 semantics like the reference) ----
    z = _rmsnorm(h1, ln2_w)
    logits = (z.astype(np.float64) @ gate_w.T.astype(np.float64)).astype(np.float32)
    order = np.argsort(-logits, axis=-1, kind="stable")
    sel = order[:, :2]                               # top-2, ties -> lower index
    vals = np.take_along_axis(logits, sel, axis=-1).astype(np.float32)
    mx = vals.max(axis=-1, keepdims=True)
    ex = np.exp(vals - mx)
    rw = (ex / ex.sum(axis=-1, keepdims=True)).astype(np.float32)

    idx_lists = []
    for e in range(E):
        m = (sel == e)
        tok = np.nonzero(m.any(axis=-1))[0]
        wgt = np.where(m, rw, 0.0).sum(axis=-1)[tok]
        idx_lists.append((tok, wgt.astype(np.float32)))
    maxload = max(len(tok) for tok, _ in idx_lists)
    cap = C
    while cap < maxload:
        cap += 512
    nc2 = _get(f"l2_{cap}", _build_l2, cap)

    # ---- launch 2: expert-parallel FFN (fp8) ----
    zT = np.clip(np.ascontiguousarray(z.T), -240, 240).astype(F8)    # [H, T]
    in_maps2 = []
    for e in range(E):
        tok, wgt = idx_lists[e]
        zeT = np.zeros((H, cap), F8)
        zeT[:, :len(tok)] = zT[:, tok]
        web = np.zeros((cap,), np.float32)
        web[:len(tok)] = wgt / (SW3 * SW2)
        in_maps2.append({
            "zeT": zeT,
            "w1T": np.clip(np.ascontiguousarray(w1[e].T) * SW1, -240, 240).astype(F8),
            "w3T": np.clip(np.ascontiguousarray(w3[e].T) * SW3, -240, 240).astype(F8),
            "w2T": np.clip(np.ascontiguousarray(w2[e].T) * SW2, -240, 240).astype(F8),
            "web": np.broadcast_to(web, (128, cap)).copy(),
        })
    res2 = run_bass_kernel_spmd(nc2, in_maps2, core_ids=list(range(NCORES)), trace=TRACE)
    LAST_RESULTS.append(res2)

    out = h1.copy()
    for e in range(E):
        tok, _ = idx_lists[e]
        out[tok] += res2.results[e]["yT"][:, :len(tok)].T.astype(np.float32)

    return out.reshape(B, S, H).astype(np.float32)



# revision 27
# speedup vs baseline: 1.7780x; 1.0240x over previous
"""Trainium2 Bass kernel for a Mixtral decoder layer (attention + top-2 MoE).

Strategy (8 NeuronCores):
  Launch 1 (attention): 2D shard = (batch b in {0,1}) x (head-group g in {0..3},
    4 heads / 256 feature slice each). Each core computes q/k/v projections for
    its slice, transposed-scores flash-style attention (scores computed as
    s^T[tk, tq] so the softmax denominator folds into a ones-column of V), and
    a partial output projection. Host sums the 4 partials per batch.
  Host: residual add, rmsnorm, gating logits, exact top-2 routing, per-expert
    token gather (expert-parallel dispatch done in numpy - free).
  Launch 2 (MoE FFN): expert-parallel - core e owns expert e's w1/w3/w2 and
    processes its routed tokens (padded to capacity C) densely, pipelined over
    512-token blocks.
  Host: scatter-add expert outputs + residual. All matmuls bf16 with fp32 PSUM
    accumulation; softmax/normalization/routing in fp32.
"""
import os
import sys

import numpy as np
import ml_dtypes

for _p in ("/root/.axon_site", "/root/.axon_site/_ro/trn_rl_repo", "/opt/trn_rl_repo"):
    if os.path.isdir(_p) and _p not in sys.path:
        sys.path.append(_p)

import concourse.tile as tile
from concourse import bacc, mybir
from concourse.bass_utils import run_bass_kernel_spmd

BF16 = ml_dtypes.bfloat16
AF = mybir.ActivationFunctionType
ALU = mybir.AluOpType
DT = mybir.dt

H = 1024
S = 2048
B = 2
NH = 16
D = 64
E = 8
I = 2048
T = B * S
EPS = 1e-5

NCORES = 8
NGRP = 4              # head groups (cores per batch)
NHPC = NH // NGRP     # 4 heads per core
DS = NHPC * D         # 256-wide feature slice per core
TQC = 4               # tq chunks of 512
NTK = S // 128        # 16 tk tiles
NCI = H // 128        # 8 contraction chunks

C = 1088              # MoE expert token capacity (per-expert max on this data ~1087)
SW1, SW3, SW2 = 64.0, 16.0, 64.0   # fp8 weight scales (powers of 2, exact to undo)
F8 = ml_dtypes.float8_e4m3

_CACHE = {}
LAST_RESULTS = []     # BassKernelResults of the last kernel() call (for test harness)
TRACE = os.environ.get("KERNEL_TRACE", "0") == "1"


def _capacity_chunks(cap):
    out, o = [], 0
    while o < cap:
        ln = min(512, cap - o)
        out.append((o, ln))
        o += ln
    return out


def _build_l1():
    """Attention, fp8-DoubleRow projections + flipped AV.

    Weights wq/wk/wv scaled by SQK=32 on host (fp8 range); q',k' = 32*true so
    scores = 1024*true, folded into the exp scale 2^-13. v' = 32*true; the
    AV output is 32*attn, normalized by the softmax denom (ones-column of v,
    unscaled), and 1/32 is folded into woT on host. AV is computed transposed:
    out[q_slice(128), d+1(65)] = pt[tk,q].T @ v[tk,65] so the denominator is a
    per-partition scalar and M=128 (full PE array)."""
    nc = bacc.Bacc("TRN2", target_bir_lowering=False, debug=False, num_devices=NCORES)
    xT8 = nc.dram_tensor("xT8", [H, S], DT.float8e4, kind="ExternalInput")
    wq8 = nc.dram_tensor("wq8", [H, DS], DT.float8e4, kind="ExternalInput")
    wk8 = nc.dram_tensor("wk8", [H, DS], DT.float8e4, kind="ExternalInput")
    wv8 = nc.dram_tensor("wv8", [H, DS], DT.float8e4, kind="ExternalInput")
    woT = nc.dram_tensor("woT", [DS, H], DT.bfloat16, kind="ExternalInput")
    h1p = nc.dram_tensor("h1p", [S, H], DT.bfloat16, kind="ExternalOutput")

    NPH = H // 256       # 4 H k-pairs for DoubleRow
    DR = mybir.MatmulPerfMode.DoubleRow
    EXPSC = 0.125 / (32.0 * 32.0)    # softmax 1/8 plus q,k weight scales
    with tile.TileContext(nc) as tc:
        with tc.tile_pool(name="wpool", bufs=1) as wpool, \
             tc.tile_pool(name="qk", bufs=1) as qkpool, \
             tc.tile_pool(name="vp", bufs=1) as vpool, \
             tc.tile_pool(name="pt", bufs=3) as ptpool, \
             tc.tile_pool(name="ao", bufs=1) as aopool, \
             tc.tile_pool(name="at", bufs=3) as atpool, \
             tc.tile_pool(name="rc", bufs=4) as rcpool, \
             tc.tile_pool(name="hout", bufs=4) as hpool, \
             tc.tile_pool(name="pp", bufs=2, space="PSUM") as pp, \
             tc.tile_pool(name="pav", bufs=1, space="PSUM") as pav, \
             tc.tile_pool(name="ppo", bufs=2, space="PSUM") as ppo:

            # ---- loads: x8/wq pairs first, wk next, wv later, wo last ----
            x8 = [[wpool.tile([128, 2, S // 2], DT.float8e4, name=f"x8_{p}_{hf}",
                              tag=f"x8_{p}_{hf}") for hf in range(2)]
                  for p in range(NPH)]
            # q/k weight tiles padded to DS+16 cols: a [.., 2, 128] slice of an
            # exactly-DS tile is fully contiguous, which walrus's LDW
            # optimization claims and then rejects for DoubleRow
            wq_t = [wpool.tile([128, 2, DS + 16], DT.float8e4, name=f"wqt{p}",
                               tag=f"wqt{p}") for p in range(NPH)]
            wk_t = [wpool.tile([128, 2, DS + 16], DT.float8e4, name=f"wkt{p}",
                               tag=f"wkt{p}") for p in range(NPH)]
            wv_t = [wpool.tile([128, 2, DS], DT.float8e4, name=f"wvt{p}",
                               tag=f"wvt{p}") for p in range(NPH)]
            xr = xT8.rearrange("(a two p) s -> a p two s", p=128, two=2)
            wqr = wq8.rearrange("(a two p) m -> a p two m", p=128, two=2)
            wkr = wk8.rearrange("(a two p) m -> a p two m", p=128, two=2)
            wvr = wv8.rearrange("(a two p) m -> a p two m", p=128, two=2)
            for p in range(NPH):
                nc.sync.dma_start(x8[p][0][:], xr[p][:, :, 0:S // 2])
                nc.sync.dma_start(wq_t[p][:, :, 0:DS], wqr[p])
            for p in range(NPH):
                nc.sync.dma_start(wk_t[p][:, :, 0:DS], wkr[p])
                nc.sync.dma_start(x8[p][1][:], xr[p][:, :, S // 2:S])
            for p in range(NPH):
                nc.sync.dma_start(wv_t[p][:], wvr[p])
            wo_sb = wpool.tile([128, DS // 128, H], DT.bfloat16)
            nc.sync.dma_start(wo_sb[:], woT.rearrange("(c p) m -> p c m", p=128))

            # q/k per head-pair [128, S] bf16 (partitions 0:64 = even head's d,
            # 64:128 = odd head's; scaled by 32); v for all heads in one
            # [128, tk-pair, 2, head, 72] fp8 tile (col 64 = ones)
            qts = [qkpool.tile([128, S], DT.bfloat16, name=f"q{p}", tag=f"q{p}")
                   for p in range(NHPC // 2)]
            kts = [qkpool.tile([128, S], DT.bfloat16, name=f"k{p}", tag=f"k{p}")
                   for p in range(NHPC // 2)]
            vall = vpool.tile([128, NTK // 2, 2, NHPC, 72], DT.float8e4)
            nc.vector.memset(vall[:, :, :, :, 64:65], 1.0)
            aoTs = [aopool.tile([128, DS // 128, S // 2], DT.bfloat16,
                                name=f"aoT{hf}", tag=f"aoT{hf}") for hf in range(2)]

            def make_qk(pair, wt, dst):
                # dst[64, 2, S] bf16 for heads (2*pair, 2*pair+1), values 32x
                for th in range(2):
                    ps = pp.tile([128, 1024], DT.float32, tag="pp", name="ps")
                    for i in range(2):
                        for p in range(NPH):
                            nc.tensor.matmul(
                                ps[:, i * 512:(i + 1) * 512],
                                wt[p][:, :, pair * 128:(pair + 1) * 128],
                                x8[p][th][:, :, i * 512:(i + 1) * 512],
                                start=(p == 0), stop=(p == NPH - 1),
                                perf_mode=DR,
                            )
                    nc.vector.tensor_copy(
                        dst[:, th * 1024:(th + 1) * 1024], ps[:, :])

            def make_v():
                for tkt in range(NTK):
                    pv = ppo.tile([128, DS], DT.float32, tag="ppo", name="pv")
                    for p in range(NPH):
                        nc.tensor.matmul(
                            pv[:, 0:DS],
                            x8[p][tkt // 8][:, :, (tkt % 8) * 128:(tkt % 8 + 1) * 128],
                            wv_t[p][:, :, :],
                            start=(p == 0), stop=(p == NPH - 1),
                            perf_mode=DR,
                        )
                    nc.vector.tensor_copy(
                        vall[:, tkt // 2, tkt % 2, :, 0:64],
                        pv[:, 0:DS].rearrange("p (h d) -> p h d", d=64))

            def av_mms(h, av, ptp, j):
                for qs in range(8):
                    nc.tensor.matmul(
                        av[:, qs, 0:65],
                        ptp[:, :, qs * 128:(qs + 1) * 128],
                        vall[:, j, :, h, 0:65],
                        start=(j == 0), stop=(j == NTK // 2 - 1),
                        perf_mode=DR,
                    )

            def attend(h, half):
                # one tq half (1024 queries, 8 slices of 128) of head h;
                # av[q_slice, qs, 0:64] = unnormalized attn (32x), [.., 64] = denom
                qt, kt = qts[h // 2], kts[h // 2]
                ro = (h % 2) * 64
                av = pav.tile([128, 8, 128], DT.float32, tag="pav", name="av")
                pending = None
                ptp = None
                for tkt in range(NTK):
                    sc = pp.tile([128, 1024], DT.float32, tag="pp", name="sc")
                    for i in range(2):
                        q0 = half * 1024 + i * 512
                        nc.tensor.matmul(
                            sc[:, i * 512:(i + 1) * 512],
                            kt[ro:ro + 64, tkt * 128:(tkt + 1) * 128],
                            qt[ro:ro + 64, q0:q0 + 512],
                            start=True, stop=True,
                        )
                    if tkt % 2 == 0:
                        ptp = ptpool.tile([128, 2, 1024], DT.float8e4, tag="pt")
                    nc.scalar.activation(ptp[:, tkt % 2, :], sc[:], AF.Exp,
                                         scale=EXPSC)
                    if tkt % 2 == 1:
                        if pending is not None:
                            av_mms(h, av, *pending)
                        pending = (ptp, tkt // 2)
                av_mms(h, av, *pending)
                # two heads of a pair share an at2 tile: cols (h%2)*64..+64;
                # transposed into aoTs after the odd head (see tp_pair)
                if h % 2 == 0:
                    at2[(h // 2, half)] = atpool.tile([128, 8, 128], DT.bfloat16,
                                                      tag="at", name="at2")
                at = at2[(h // 2, half)]
                roff = (h % 2) * 64
                rec = rcpool.tile([128, 8], DT.float32, tag="rc")
                nc.vector.reciprocal(rec[:, :], av[:, :, 64])
                nc.vector.tensor_tensor(
                    at[:, :, roff:roff + 64], av[:, :, 0:64],
                    rec[:, :, None].to_broadcast([128, 8, 64]), ALU.mult)

            def tp_pair(pair, half):
                at = at2.pop((pair, half))
                for qs in range(8):
                    nc.sync.dma_start_transpose(
                        aoTs[half][:, pair, qs * 128:(qs + 1) * 128],
                        at[:, qs, :])

            def oproj(half, ts):
                # h1p[tq, :] partial for the given tq tiles of this half
                for t in ts:
                    tkt = half * 8 + t
                    ht = hpool.tile([128, H], DT.bfloat16, tag="ht")
                    for jc in range(2):
                        po = ppo.tile([128, 512], DT.float32, tag="ppo", name="po")
                        for oc in range(2):
                            nc.tensor.matmul(
                                po[:, :],
                                aoTs[half][:, oc, t * 128:(t + 1) * 128],
                                wo_sb[:, oc, jc * 512:(jc + 1) * 512],
                                start=(oc == 0), stop=(oc == 1),
                            )
                        nc.vector.tensor_copy(ht[:, jc * 512:(jc + 1) * 512], po[:])
                    nc.sync.dma_start(h1p[tkt * 128:(tkt + 1) * 128, :], ht[:])

            at2 = {}
            make_qk(0, wq_t, qts[0])
            make_qk(0, wk_t, kts[0])
            make_qk(1, wq_t, qts[1])
            make_qk(1, wk_t, kts[1])
            make_v()
            attend(0, 0)
            attend(1, 0)
            tp_pair(0, 0)
            attend(2, 0)
            attend(3, 0)
            tp_pair(1, 0)
            # oproj(0) interleaved with half-1 attends so PE never displaces
            # the ACT-critical scores for long
            attend(0, 1)
            oproj(0, range(0, 3))
            attend(1, 1)
            oproj(0, range(3, 6))
            tp_pair(0, 1)
            attend(2, 1)
            oproj(0, range(6, 8))
            attend(3, 1)
            tp_pair(1, 1)
            oproj(1, range(8))

    nc.compile()
    nc.finalize()
    return nc


def _build_l2(cap):
    """Expert-parallel MoE FFN, fp8e4m3 + DoubleRow (2 K-tiles/instruction).

    Scales: w1 *= SW1 (silu scale=1/SW1 undoes exactly), w3 *= SW3,
    w2 *= SW2; web divided by SW3*SW2 on host. hh = silu_true * SW3*g_true
    stays well under fp8e4 max 240 for SW3=16."""
    nc = bacc.Bacc("TRN2", target_bir_lowering=False, debug=False, num_devices=NCORES)
    zeT = nc.dram_tensor("zeT", [H, cap], DT.float8e4, kind="ExternalInput")
    w1T = nc.dram_tensor("w1T", [H, I], DT.float8e4, kind="ExternalInput")
    w3T = nc.dram_tensor("w3T", [H, I], DT.float8e4, kind="ExternalInput")
    w2T = nc.dram_tensor("w2T", [I, H], DT.float8e4, kind="ExternalInput")
    web = nc.dram_tensor("web", [128, cap], DT.float32, kind="ExternalInput")
    yT = nc.dram_tensor("yT", [H, cap], DT.bfloat16, kind="ExternalOutput")

    cch = _capacity_chunks(cap)
    NIC = I // 128
    NPH = H // 256        # 4 K-pairs over H
    NPI = I // 256        # 8 K-pairs over I
    DR = mybir.MatmulPerfMode.DoubleRow
    with tile.TileContext(nc) as tc:
        with tc.tile_pool(name="wpool", bufs=1) as wpool, \
             tc.tile_pool(name="hh", bufs=1) as hhpool, \
             tc.tile_pool(name="hs", bufs=3) as hspool, \
             tc.tile_pool(name="yt", bufs=3) as ytpool, \
             tc.tile_pool(name="pg", bufs=4, space="PSUM") as pg, \
             tc.tile_pool(name="py", bufs=4, space="PSUM") as py:

            # K-pair tiles; z split per cap-chunk and w1 per 4-ic block so the
            # first accumulation group starts after ~2 small DMAs
            zps = [[wpool.tile([128, 2, ln], DT.float8e4, name=f"zp{p}_{j}",
                               tag=f"zp{p}_{j}") for j, (o, ln) in enumerate(cch)]
                   for p in range(NPH)]
            w1ps = [[wpool.tile([128, 2, 512], DT.float8e4, name=f"w1p{p}_{b}",
                                tag=f"w1p{p}_{b}") for b in range(4)]
                    for p in range(NPH)]
            w3ps = [[wpool.tile([128, 2, 512], DT.float8e4, name=f"w3p{p}_{b}",
                                tag=f"w3p{p}_{b}") for b in range(4)]
                    for p in range(NPH)]
            zr = zeT.rearrange("(a two p) m -> a p two m", p=128, two=2)
            w1r = w1T.rearrange("(a two p) m -> a p two m", p=128, two=2)
            w3r = w3T.rearrange("(a two p) m -> a p two m", p=128, two=2)
            for j, (o, ln) in enumerate(cch):
                for p in range(NPH):
                    nc.sync.dma_start(zps[p][j][:], zr[p][:, :, o:o + ln])
                    if j == 0:
                        nc.sync.dma_start(w1ps[p][0][:], w1r[p][:, :, 0:512])
            for b in range(1, 4):
                for p in range(NPH):
                    nc.sync.dma_start(w1ps[p][b][:], w1r[p][:, :, b * 512:(b + 1) * 512])
            for b in range(4):
                for p in range(NPH):
                    nc.sync.dma_start(w3ps[p][b][:], w3r[p][:, :, b * 512:(b + 1) * 512])
            web_sb = wpool.tile([128, cap], DT.float32)
            nc.sync.dma_start(web_sb[:], web[:, :])

            # hh as K-pair tiles over I for the DoubleRow y-phase
            hhp = [hhpool.tile([128, 2, cap], DT.float8e4, name=f"hhp{p}",
                               tag=f"hhp{p}") for p in range(NPI)]
            w2_holder = []

            for ic in range(NIC):
                b, bo = ic // 4, (ic % 4) * 128
                hs = hspool.tile([128, cap], DT.float8e4, tag="hs", name="hs")
                for j, (o, ln) in enumerate(cch):
                    hp = pg.tile([128, 512], DT.float32, tag="pg", name="hp")
                    for p in range(NPH):
                        nc.tensor.matmul(
                            hp[:, 0:ln],
                            w1ps[p][b][:, :, bo:bo + 128],
                            zps[p][j][:, :, 0:ln],
                            start=(p == 0), stop=(p == NPH - 1),
                            perf_mode=DR,
                        )
                    nc.scalar.activation(hs[:, o:o + ln], hp[:, 0:ln], AF.Silu,
                                         scale=1.0 / SW1)
                    gp = pg.tile([128, 512], DT.float32, tag="pg", name="gp")
                    for p in range(NPH):
                        nc.tensor.matmul(
                            gp[:, 0:ln],
                            w3ps[p][b][:, :, bo:bo + 128],
                            zps[p][j][:, :, 0:ln],
                            start=(p == 0), stop=(p == NPH - 1),
                            perf_mode=DR,
                        )
                    nc.vector.tensor_tensor(
                        hhp[ic // 2][:, ic % 2, o:o + ln],
                        gp[:, 0:ln], hs[:, o:o + ln], ALU.mult)
                if ic == 0:
                    # emit w2 load after the first h-block for DMA priority
                    w2ps = [wpool.tile([128, 2, H], DT.float8e4, name=f"w2p{p}",
                                       tag=f"w2p{p}") for p in range(NPI)]
                    w2r = w2T.rearrange("(a two p) m -> a p two m", p=128, two=2)
                    for p in range(NPI):
                        nc.sync.dma_start(w2ps[p][:], w2r[p])
                    w2_holder.append(w2ps)

            w2ps = w2_holder[0]
            for hc in range(NCI):
                yt = ytpool.tile([128, cap], DT.bfloat16, tag="yt", name="yt")
                for j, (o, ln) in enumerate(cch):
                    yp = py.tile([128, 512], DT.float32, tag="py", name="yp")
                    for p in range(NPI):
                        nc.tensor.matmul(
                            yp[:, 0:ln],
                            w2ps[p][:, :, hc * 128:(hc + 1) * 128],
                            hhp[p][:, :, o:o + ln],
                            start=(p == 0), stop=(p == NPI - 1),
                            perf_mode=DR,
                        )
                    nc.vector.tensor_tensor(
                        yt[:, o:o + ln], yp[:, 0:ln], web_sb[:, o:o + ln], ALU.mult)
                nc.sync.dma_start(yT[hc * 128:(hc + 1) * 128, :], yt[:])

    nc.compile()
    nc.finalize()
    return nc


def _get(name, builder, *args):
    if name not in _CACHE:
        _CACHE[name] = builder(*args)
    return _CACHE[name]


def _rmsnorm(x, w):
    xf = x.astype(np.float32)
    rms = 1.0 / np.sqrt((xf * xf).mean(axis=-1, keepdims=True) + EPS)
    return (xf * rms) * w.astype(np.float32)


def kernel(x, ln1_w, ln2_w, wq, wk, wv, wo, gate_w, w1, w2, w3):
    global LAST_RESULTS
    LAST_RESULTS = []
    x = np.asarray(x, np.float32)
    wq, wk, wv, wo = (np.asarray(a, np.float32) for a in (wq, wk, wv, wo))
    gate_w = np.asarray(gate_w, np.float32)
    w1, w2, w3 = (np.asarray(a, np.float32) for a in (w1, w2, w3))
    ln1_w = np.asarray(ln1_w, np.float32)
    ln2_w = np.asarray(ln2_w, np.float32)

    xf = x.reshape(T, H)
    z1 = _rmsnorm(xf, ln1_w)
    # ---- launch 1: attention (fp8 projections, bf16 scores/O-proj) ----
    SQK = 32.0
    nc1 = _get("l1", _build_l1)
    z1_8 = np.clip(z1, -240, 240).astype(F8)
    in_maps = []
    for c in range(NCORES):
        b, g = divmod(c, NGRP)
        sl = slice(g * DS, (g + 1) * DS)
        in_maps.append({
            "xT8": np.ascontiguousarray(z1_8[b * S:(b + 1) * S].T),
            "wq8": np.clip(np.ascontiguousarray(wq[sl].T) * SQK, -240, 240).astype(F8),
            "wk8": np.clip(np.ascontiguousarray(wk[sl].T) * SQK, -240, 240).astype(F8),
            "wv8": np.clip(np.ascontiguousarray(wv[sl].T) * SQK, -240, 240).astype(F8),
            "woT": (np.ascontiguousarray(wo[:, sl].T) / SQK).astype(BF16),
        })
    res1 = run_bass_kernel_spmd(nc1, in_maps, core_ids=list(range(NCORES)), trace=TRACE)
    LAST_RESULTS.append(res1)

    h1 = xf.copy()
    for c in range(NCORES):
        b = c // NGRP
        h1[b * S:(b + 1) * S] += res1.results[c]["h1p"].astype(np.float32)

    # ---- host: routing (exact fp32 semantics like the reference) ----
    z = _rmsnorm(h1, ln2_w)
    logits = (z.astype(np.float64) @ gate_w.T.astype(np.float64)).astype(np.float32)
    order = np.argsort(-logits, axis=-1, kind="stable")
    sel = order[:, :2]                               # top-2, ties -> lower index
    vals = np.take_along_axis(logits, sel, axis=-1).astype(np.float32)
    mx = vals.max(axis=-1, keepdims=True)
    ex = np.exp(vals - mx)
    rw = (ex / ex.sum(axis=-1, keepdims=True)).astype(np.float32)

    idx_lists = []
    for e in range(E):
        m = (sel == e)
        tok = np.nonzero(m.any(axis=-1))[0]
        wgt = np.where(m, rw, 0.0).sum(axis=-1)[tok]
        idx_lists.append((tok, wgt.astype(np.float32)))
    maxload = max(len(tok) for tok, _ in idx_lists)
    cap = C
    while cap < maxload:
        cap += 512
    nc2 = _get(f"l2_{cap}", _build_l2, cap)

    # ---- launch 2: expert-parallel FFN (fp8) ----
    zT = np.clip(np.ascontiguousarray(z.T), -240, 240).astype(F8)    # [H, T]
    in_maps2 = []
    for e in range(E):
        tok, wgt = idx_lists[e]
        zeT = np.zeros((H, cap), F8)
        zeT[:, :len(tok)] = zT[:, tok]
        web = np.zeros((cap,), np.float32)
        web[:len(tok)] = wgt / (SW3 * SW2)
        in_maps2.append({
            "zeT": zeT,
            "w1T": np.clip(np.ascontiguousarray(w1[e].T) * SW1, -240, 240).astype(F8),
            "w3T": np.clip(np.ascontiguousarray(w3[e].T) * SW3, -240, 240).astype(F8),
            "w2T": np.clip(np.ascontiguousarray(w2[e].T) * SW2, -240, 240).astype(F8),
            "web": np.broadcast_to(web, (128, cap)).copy(),
        })
    res2 = run_bass_kernel_spmd(nc2, in_maps2, core_ids=list(range(NCORES)), trace=TRACE)
    LAST_RESULTS.append(res2)

    out = h1.copy()
    for e in range(E):
        tok, _ = idx_lists[e]
        out[tok] += res2.results[e]["yT"][:, :len(tok)].T.astype(np.float32)

    return out.reshape(B, S, H).astype(np.float32)



# revision 33
# speedup vs baseline: 1.9124x; 1.0756x over previous
"""Trainium2 Bass kernel for a Mixtral decoder layer (attention + top-2 MoE).

Strategy (8 NeuronCores):
  Launch 1 (attention): 2D shard = (batch b in {0,1}) x (head-group g in {0..3},
    4 heads / 256 feature slice each). Each core computes q/k/v projections for
    its slice, transposed-scores flash-style attention (scores computed as
    s^T[tk, tq] so the softmax denominator folds into a ones-column of V), and
    a partial output projection. Host sums the 4 partials per batch.
  Host: residual add, rmsnorm, gating logits, exact top-2 routing, per-expert
    token gather (expert-parallel dispatch done in numpy - free).
  Launch 2 (MoE FFN): expert-parallel - core e owns expert e's w1/w3/w2 and
    processes its routed tokens (padded to capacity C) densely, pipelined over
    512-token blocks.
  Host: scatter-add expert outputs + residual. All matmuls bf16 with fp32 PSUM
    accumulation; softmax/normalization/routing in fp32.
"""
import os
import sys

import numpy as np
import ml_dtypes

for _p in ("/root/.axon_site", "/root/.axon_site/_ro/trn_rl_repo", "/opt/trn_rl_repo"):
    if os.path.isdir(_p) and _p not in sys.path:
        sys.path.append(_p)

import concourse.tile as tile
from concourse import bacc, mybir
from concourse.bass_utils import run_bass_kernel_spmd

BF16 = ml_dtypes.bfloat16
AF = mybir.ActivationFunctionType
ALU = mybir.AluOpType
DT = mybir.dt

H = 1024
S = 2048
B = 2
NH = 16
D = 64
E = 8
I = 2048
T = B * S
EPS = 1e-5

NCORES = 8
NGRP = 4              # head groups (cores per batch)
NHPC = NH // NGRP     # 4 heads per core
DS = NHPC * D         # 256-wide feature slice per core
TQC = 4               # tq chunks of 512
NTK = S // 128        # 16 tk tiles
NCI = H // 128        # 8 contraction chunks

C = 1088              # MoE expert token capacity (per-expert max on this data ~1087)
SW1, SW3, SW2 = 64.0, 16.0, 64.0   # fp8 weight scales (powers of 2, exact to undo)
F8 = ml_dtypes.float8_e4m3

_CACHE = {}
LAST_RESULTS = []     # BassKernelResults of the last kernel() call (for test harness)
TRACE = os.environ.get("KERNEL_TRACE", "0") == "1"

# tk tiles per attend-half whose softmax exp runs on DVE (custom fused op)
# instead of ACT, to balance the two engines
DVE_EXP_TKT = (5, 10, 15)


def _register_exp_ops():
    """Register two custom DVE ops computing exp via a degree-2 polynomial
    base and repeated squaring: op1 = (1 + w + w^2/2)^8 ~ e^{8w} with
    w = in*C0, op2 = x^4. Chained with C0 = scale/32 they give e^{scale*in}
    to ~1% relative accuracy on |scale*in| <= 4.2."""
    import concourse.dve_ops as dve_ops
    from concourse.dve_spec import Spec, Src0, C0, C1, One, sq, lower
    from concourse.dve_spec import _has_src1 as has_src1
    from concourse.dve_ops import DveOp, OPS, CUSTOM_DVE_SPECS, _SUB_OPCODE_FOR_NAME
    from concourse.dve_uop import DveOpSpec

    if "EXP_BASE_ANT" in CUSTOM_DVE_SPECS:
        return dve_ops.EXP_BASE_ANT, dve_ops.EXP_SQ2_ANT

    w = Src0 * C0
    p = One + w * (One + w * C1)

    def ref1(in0, in1, c0, c1, c2):
        ww = in0.astype(np.float32) * c0
        pp = 1.0 + ww * (1.0 + ww * c1)
        return (pp ** 8).astype(np.float32)

    def ref2(in0, in1, c0, c1, c2):
        x = in0.astype(np.float32)
        return (x * x) * (x * x)

    ops = []
    row = max(_SUB_OPCODE_FOR_NAME.values()) + 1
    for name, body, ref in (("EXP_BASE_ANT", sq(sq(sq(p))), ref1),
                            ("EXP_SQ2_ANT", sq(sq(Src0)), ref2)):
        spec = Spec(body=body, reference=ref)
        shas = {}
        for ver in ("v3", "v4"):
            s = DveOpSpec(name=name, opcode=row, uops=lower(spec, ver=ver),
                          rd1_en=has_src1(spec))
            shas[ver] = s.sha(ver)
        op = DveOp(name, spec, subdim=False, uops_sha=shas)
        _SUB_OPCODE_FOR_NAME[name] = row
        OPS.append(op)
        CUSTOM_DVE_SPECS[name] = spec
        setattr(dve_ops, name, op)
        ops.append(op)
        row += 1
    assert row <= 0x20
    return ops[0], ops[1]


def _capacity_chunks(cap):
    out, o = [], 0
    while o < cap:
        ln = min(512, cap - o)
        out.append((o, ln))
        o += ln
    return out


def _build_l1():
    """Attention, fp8-DoubleRow projections + flipped AV.

    Weights wq/wk/wv scaled by SQK=32 on host (fp8 range); q',k' = 32*true so
    scores = 1024*true, folded into the exp scale 2^-13. v' = 32*true; the
    AV output is 32*attn, normalized by the softmax denom (ones-column of v,
    unscaled), and 1/32 is folded into woT on host. AV is computed transposed:
    out[q_slice(128), d+1(65)] = pt[tk,q].T @ v[tk,65] so the denominator is a
    per-partition scalar and M=128 (full PE array)."""
    nc = bacc.Bacc("TRN2", target_bir_lowering=False, debug=False, num_devices=NCORES)
    xT8 = nc.dram_tensor("xT8", [H, S], DT.float8e4, kind="ExternalInput")
    wq8 = nc.dram_tensor("wq8", [H, DS], DT.float8e4, kind="ExternalInput")
    wk8 = nc.dram_tensor("wk8", [H, DS], DT.float8e4, kind="ExternalInput")
    wv8 = nc.dram_tensor("wv8", [H, DS], DT.float8e4, kind="ExternalInput")
    woT = nc.dram_tensor("woT", [DS, H], DT.bfloat16, kind="ExternalInput")
    h1p = nc.dram_tensor("h1p", [S, H], DT.bfloat16, kind="ExternalOutput")

    NPH = H // 256       # 4 H k-pairs for DoubleRow
    DR = mybir.MatmulPerfMode.DoubleRow
    EXPSC = 0.125 / (32.0 * 32.0)    # softmax 1/8 plus q,k weight scales
    EXP_BASE, EXP_SQ2 = _register_exp_ops()
    with tile.TileContext(nc) as tc:
        with tc.tile_pool(name="wpool", bufs=1) as wpool, \
             tc.tile_pool(name="qk", bufs=1) as qkpool, \
             tc.tile_pool(name="vp", bufs=1) as vpool, \
             tc.tile_pool(name="pt", bufs=3) as ptpool, \
             tc.tile_pool(name="ao", bufs=1) as aopool, \
             tc.tile_pool(name="at", bufs=3) as atpool, \
             tc.tile_pool(name="rc", bufs=4) as rcpool, \
             tc.tile_pool(name="st", bufs=2) as stpool, \
             tc.tile_pool(name="hout", bufs=4) as hpool, \
             tc.tile_pool(name="pp", bufs=2, space="PSUM") as pp, \
             tc.tile_pool(name="pav", bufs=1, space="PSUM") as pav, \
             tc.tile_pool(name="ppo", bufs=2, space="PSUM") as ppo:

            # ---- loads: x8/wq pairs first, wk next, wv later, wo last ----
            x8 = [[wpool.tile([128, 2, S // 2], DT.float8e4, name=f"x8_{p}_{hf}",
                              tag=f"x8_{p}_{hf}") for hf in range(2)]
                  for p in range(NPH)]
            # q/k weight tiles padded to DS+16 cols: a [.., 2, 128] slice of an
            # exactly-DS tile is fully contiguous, which walrus's LDW
            # optimization claims and then rejects for DoubleRow
            wq_t = [wpool.tile([128, 2, DS + 16], DT.float8e4, name=f"wqt{p}",
                               tag=f"wqt{p}") for p in range(NPH)]
            wk_t = [wpool.tile([128, 2, DS + 16], DT.float8e4, name=f"wkt{p}",
                               tag=f"wkt{p}") for p in range(NPH)]
            wv_t = [wpool.tile([128, 2, DS], DT.float8e4, name=f"wvt{p}",
                               tag=f"wvt{p}") for p in range(NPH)]
            xr = xT8.rearrange("(a two p) s -> a p two s", p=128, two=2)
            wqr = wq8.rearrange("(a two p) m -> a p two m", p=128, two=2)
            wkr = wk8.rearrange("(a two p) m -> a p two m", p=128, two=2)
            wvr = wv8.rearrange("(a two p) m -> a p two m", p=128, two=2)
            for p in range(NPH):
                nc.sync.dma_start(x8[p][0][:], xr[p][:, :, 0:S // 2])
                nc.sync.dma_start(wq_t[p][:, :, 0:DS], wqr[p])
            for p in range(NPH):
                nc.sync.dma_start(wk_t[p][:, :, 0:DS], wkr[p])
                nc.sync.dma_start(x8[p][1][:], xr[p][:, :, S // 2:S])
            for p in range(NPH):
                nc.sync.dma_start(wv_t[p][:], wvr[p])
            wo_sb = wpool.tile([128, DS // 128, H], DT.bfloat16)
            nc.sync.dma_start(wo_sb[:], woT.rearrange("(c p) m -> p c m", p=128))

            # q/k per head-pair [128, S] bf16 (partitions 0:64 = even head's d,
            # 64:128 = odd head's; scaled by 32); v for all heads in one
            # [128, tk-pair, 2, head, 72] fp8 tile (col 64 = ones)
            qts = [qkpool.tile([128, S], DT.bfloat16, name=f"q{p}", tag=f"q{p}")
                   for p in range(NHPC // 2)]
            kts = [qkpool.tile([128, S], DT.bfloat16, name=f"k{p}", tag=f"k{p}")
                   for p in range(NHPC // 2)]
            vall = vpool.tile([128, NTK // 2, 2, NHPC, 72], DT.float8e4)
            nc.vector.memset(vall[:, :, :, :, 64:65], 1.0)
            aoTs = [aopool.tile([128, DS // 128, S // 2], DT.bfloat16,
                                name=f"aoT{hf}", tag=f"aoT{hf}") for hf in range(2)]

            def make_qk(pair, wt, dst):
                # dst[64, 2, S] bf16 for heads (2*pair, 2*pair+1), values 32x
                for th in range(2):
                    ps = pp.tile([128, 1024], DT.float32, tag="pp", name="ps")
                    for i in range(2):
                        for p in range(NPH):
                            nc.tensor.matmul(
                                ps[:, i * 512:(i + 1) * 512],
                                wt[p][:, :, pair * 128:(pair + 1) * 128],
                                x8[p][th][:, :, i * 512:(i + 1) * 512],
                                start=(p == 0), stop=(p == NPH - 1),
                                perf_mode=DR,
                            )
                    nc.vector.tensor_copy(
                        dst[:, th * 1024:(th + 1) * 1024], ps[:, :])

            def make_v():
                for tkt in range(NTK):
                    pv = ppo.tile([128, DS], DT.float32, tag="ppo", name="pv")
                    for p in range(NPH):
                        nc.tensor.matmul(
                            pv[:, 0:DS],
                            x8[p][tkt // 8][:, :, (tkt % 8) * 128:(tkt % 8 + 1) * 128],
                            wv_t[p][:, :, :],
                            start=(p == 0), stop=(p == NPH - 1),
                            perf_mode=DR,
                        )
                    nc.vector.tensor_copy(
                        vall[:, tkt // 2, tkt % 2, :, 0:64],
                        pv[:, 0:DS].rearrange("p (h d) -> p h d", d=64))

            def av_mms(h, av, ptp, j):
                for qs in range(8):
                    nc.tensor.matmul(
                        av[:, qs, 0:65],
                        ptp[:, :, qs * 128:(qs + 1) * 128],
                        vall[:, j, :, h, 0:65],
                        start=(j == 0), stop=(j == NTK // 2 - 1),
                        perf_mode=DR,
                    )

            def attend(h, half):
                # one tq half (1024 queries, 8 slices of 128) of head h;
                # av[q_slice, qs, 0:64] = unnormalized attn (32x), [.., 64] = denom
                qt, kt = qts[h // 2], kts[h // 2]
                ro = (h % 2) * 64
                av = pav.tile([128, 8, 128], DT.float32, tag="pav", name="av")
                pending = None
                ptp = None
                for tkt in range(NTK):
                    sc = pp.tile([128, 1024], DT.float32, tag="pp", name="sc")
                    for i in range(2):
                        q0 = half * 1024 + i * 512
                        nc.tensor.matmul(
                            sc[:, i * 512:(i + 1) * 512],
                            kt[ro:ro + 64, tkt * 128:(tkt + 1) * 128],
                            qt[ro:ro + 64, q0:q0 + 512],
                            start=True, stop=True,
                        )
                    if tkt % 2 == 0:
                        ptp = ptpool.tile([128, 2, 1024], DT.float8e4, tag="pt")
                    if tkt in DVE_EXP_TKT:
                        st = stpool.tile([128, 1024], DT.bfloat16, tag="st")
                        nc.vector._custom_dve(EXP_BASE, out=st[:], in0=sc[:],
                                              s0=EXPSC / 32.0, s1=0.5)
                        nc.vector._custom_dve(EXP_SQ2, out=ptp[:, tkt % 2, :],
                                              in0=st[:])
                    else:
                        nc.scalar.activation(ptp[:, tkt % 2, :], sc[:], AF.Exp,
                                             scale=EXPSC)
                    if tkt % 2 == 1:
                        if pending is not None:
                            av_mms(h, av, *pending)
                        pending = (ptp, tkt // 2)
                av_mms(h, av, *pending)
                # two heads of a pair share an at2 tile: cols (h%2)*64..+64;
                # transposed into aoTs after the odd head (see tp_pair)
                if h % 2 == 0:
                    at2[(h // 2, half)] = atpool.tile([128, 8, 128], DT.bfloat16,
                                                      tag="at", name="at2")
                at = at2[(h // 2, half)]
                roff = (h % 2) * 64
                rec = rcpool.tile([128, 8], DT.float32, tag="rc")
                nc.vector.reciprocal(rec[:, :], av[:, :, 64])
                nc.vector.tensor_tensor(
                    at[:, :, roff:roff + 64], av[:, :, 0:64],
                    rec[:, :, None].to_broadcast([128, 8, 64]), ALU.mult)

            def tp_pair(pair, half):
                at = at2.pop((pair, half))
                for qs in range(8):
                    nc.sync.dma_start_transpose(
                        aoTs[half][:, pair, qs * 128:(qs + 1) * 128],
                        at[:, qs, :])

            def oproj(half, ts):
                # h1p[tq, :] partial for the given tq tiles of this half
                for t in ts:
                    tkt = half * 8 + t
                    ht = hpool.tile([128, H], DT.bfloat16, tag="ht")
                    for jc in range(2):
                        po = ppo.tile([128, 512], DT.float32, tag="ppo", name="po")
                        for oc in range(2):
                            nc.tensor.matmul(
                                po[:, :],
                                aoTs[half][:, oc, t * 128:(t + 1) * 128],
                                wo_sb[:, oc, jc * 512:(jc + 1) * 512],
                                start=(oc == 0), stop=(oc == 1),
                            )
                        nc.vector.tensor_copy(ht[:, jc * 512:(jc + 1) * 512], po[:])
                    nc.sync.dma_start(h1p[tkt * 128:(tkt + 1) * 128, :], ht[:])

            at2 = {}
            make_qk(0, wq_t, qts[0])
            make_qk(0, wk_t, kts[0])
            make_qk(1, wq_t, qts[1])
            make_qk(1, wk_t, kts[1])
            make_v()
            attend(0, 0)
            attend(1, 0)
            tp_pair(0, 0)
            attend(2, 0)
            attend(3, 0)
            tp_pair(1, 0)
            # oproj(0) interleaved with half-1 attends so PE never displaces
            # the ACT-critical scores for long
            attend(0, 1)
            oproj(0, range(0, 3))
            attend(1, 1)
            oproj(0, range(3, 6))
            tp_pair(0, 1)
            attend(2, 1)
            oproj(0, range(6, 8))
            attend(3, 1)
            tp_pair(1, 1)
            oproj(1, range(8))

    nc.compile()
    nc.finalize()
    return nc


def _build_l2(cap):
    """Expert-parallel MoE FFN, fp8e4m3 + DoubleRow (2 K-tiles/instruction).

    Scales: w1 *= SW1 (silu scale=1/SW1 undoes exactly), w3 *= SW3,
    w2 *= SW2; web divided by SW3*SW2 on host. hh = silu_true * SW3*g_true
    stays well under fp8e4 max 240 for SW3=16."""
    nc = bacc.Bacc("TRN2", target_bir_lowering=False, debug=False, num_devices=NCORES)
    zeT = nc.dram_tensor("zeT", [H, cap], DT.float8e4, kind="ExternalInput")
    w1T = nc.dram_tensor("w1T", [H, I], DT.float8e4, kind="ExternalInput")
    w3T = nc.dram_tensor("w3T", [H, I], DT.float8e4, kind="ExternalInput")
    w2T = nc.dram_tensor("w2T", [I, H], DT.float8e4, kind="ExternalInput")
    web = nc.dram_tensor("web", [128, cap], DT.float32, kind="ExternalInput")
    yT = nc.dram_tensor("yT", [H, cap], DT.bfloat16, kind="ExternalOutput")

    cch = _capacity_chunks(cap)
    NIC = I // 128
    NPH = H // 256        # 4 K-pairs over H
    NPI = I // 256        # 8 K-pairs over I
    DR = mybir.MatmulPerfMode.DoubleRow
    with tile.TileContext(nc) as tc:
        with tc.tile_pool(name="wpool", bufs=1) as wpool, \
             tc.tile_pool(name="hh", bufs=1) as hhpool, \
             tc.tile_pool(name="hs", bufs=3) as hspool, \
             tc.tile_pool(name="yt", bufs=3) as ytpool, \
             tc.tile_pool(name="pg", bufs=4, space="PSUM") as pg, \
             tc.tile_pool(name="py", bufs=4, space="PSUM") as py:

            # K-pair tiles; w1/w3 split in I-halves, loads interleaved per
            # pair so the h-phase streams behind the DMAs
            zps = [wpool.tile([128, 2, cap], DT.float8e4, name=f"zp{p}",
                              tag=f"zp{p}") for p in range(NPH)]
            w1ps = [[wpool.tile([128, 2, I // 2], DT.float8e4, name=f"w1p{p}_{b}",
                                tag=f"w1p{p}_{b}") for b in range(2)]
                    for p in range(NPH)]
            w3ps = [[wpool.tile([128, 2, I // 2], DT.float8e4, name=f"w3p{p}_{b}",
                                tag=f"w3p{p}_{b}") for b in range(2)]
                    for p in range(NPH)]
            zr = zeT.rearrange("(a two p) m -> a p two m", p=128, two=2)
            w1r = w1T.rearrange("(a two p) m -> a p two m", p=128, two=2)
            w3r = w3T.rearrange("(a two p) m -> a p two m", p=128, two=2)
            for p in range(NPH):
                nc.sync.dma_start(zps[p][:], zr[p])
                nc.sync.dma_start(w1ps[p][0][:], w1r[p][:, :, 0:I // 2])
                nc.sync.dma_start(w3ps[p][0][:], w3r[p][:, :, 0:I // 2])
            for p in range(NPH):
                nc.sync.dma_start(w1ps[p][1][:], w1r[p][:, :, I // 2:I])
                nc.sync.dma_start(w3ps[p][1][:], w3r[p][:, :, I // 2:I])
            web_sb = wpool.tile([128, cap], DT.float32)
            nc.sync.dma_start(web_sb[:], web[:, :])

            # hh as K-pair tiles over I for the DoubleRow y-phase
            hhp = [hhpool.tile([128, 2, cap], DT.float8e4, name=f"hhp{p}",
                               tag=f"hhp{p}") for p in range(NPI)]
            w2_holder = []

            for ic in range(NIC):
                b, bo = ic // 8, (ic % 8) * 128
                hs = hspool.tile([128, cap], DT.float8e4, tag="hs", name="hs")
                for j, (o, ln) in enumerate(cch):
                    hp = pg.tile([128, 512], DT.float32, tag="pg", name="hp")
                    for p in range(NPH):
                        nc.tensor.matmul(
                            hp[:, 0:ln],
                            w1ps[p][b][:, :, bo:bo + 128],
                            zps[p][:, :, o:o + ln],
                            start=(p == 0), stop=(p == NPH - 1),
                            perf_mode=DR,
                        )
                    nc.scalar.activation(hs[:, o:o + ln], hp[:, 0:ln], AF.Silu,
                                         scale=1.0 / SW1)
                    gp = pg.tile([128, 512], DT.float32, tag="pg", name="gp")
                    for p in range(NPH):
                        nc.tensor.matmul(
                            gp[:, 0:ln],
                            w3ps[p][b][:, :, bo:bo + 128],
                            zps[p][:, :, o:o + ln],
                            start=(p == 0), stop=(p == NPH - 1),
                            perf_mode=DR,
                        )
                    nc.vector.tensor_tensor(
                        hhp[ic // 2][:, ic % 2, o:o + ln],
                        gp[:, 0:ln], hs[:, o:o + ln], ALU.mult)
                if ic == 0:
                    # emit w2 load after the first h-block for DMA priority
                    w2ps = [wpool.tile([128, 2, H], DT.float8e4, name=f"w2p{p}",
                                       tag=f"w2p{p}") for p in range(NPI)]
                    w2r = w2T.rearrange("(a two p) m -> a p two m", p=128, two=2)
                    for p in range(NPI):
                        nc.sync.dma_start(w2ps[p][:], w2r[p])
                    w2_holder.append(w2ps)

            w2ps = w2_holder[0]
            for hc in range(NCI):
                yt = ytpool.tile([128, cap], DT.bfloat16, tag="yt", name="yt")
                for j, (o, ln) in enumerate(cch):
                    yp = py.tile([128, 512], DT.float32, tag="py", name="yp")
                    for p in range(NPI):
                        nc.tensor.matmul(
                            yp[:, 0:ln],
                            w2ps[p][:, :, hc * 128:(hc + 1) * 128],
                            hhp[p][:, :, o:o + ln],
                            start=(p == 0), stop=(p == NPI - 1),
                            perf_mode=DR,
                        )
                    nc.vector.tensor_tensor(
                        yt[:, o:o + ln], yp[:, 0:ln], web_sb[:, o:o + ln], ALU.mult)
                nc.sync.dma_start(yT[hc * 128:(hc + 1) * 128, :], yt[:])

    nc.compile()
    nc.finalize()
    return nc


def _get(name, builder, *args):
    if name not in _CACHE:
        _CACHE[name] = builder(*args)
    return _CACHE[name]


def _rmsnorm(x, w):
    xf = x.astype(np.float32)
    rms = 1.0 / np.sqrt((xf * xf).mean(axis=-1, keepdims=True) + EPS)
    return (xf * rms) * w.astype(np.float32)


def kernel(x, ln1_w, ln2_w, wq, wk, wv, wo, gate_w, w1, w2, w3):
    global LAST_RESULTS
    LAST_RESULTS = []
    x = np.asarray(x, np.float32)
    wq, wk, wv, wo = (np.asarray(a, np.float32) for a in (wq, wk, wv, wo))
    gate_w = np.asarray(gate_w, np.float32)
    w1, w2, w3 = (np.asarray(a, np.float32) for a in (w1, w2, w3))
    ln1_w = np.asarray(ln1_w, np.float32)
    ln2_w = np.asarray(ln2_w, np.float32)

    xf = x.reshape(T, H)
    z1 = _rmsnorm(xf, ln1_w)
    # ---- launch 1: attention (fp8 projections, bf16 scores/O-proj) ----
    SQK = 32.0
    nc1 = _get("l1", _build_l1)
    z1_8 = np.clip(z1, -240, 240).astype(F8)
    in_maps = []
    for c in range(NCORES):
        b, g = divmod(c, NGRP)
        sl = slice(g * DS, (g + 1) * DS)
        in_maps.append({
            "xT8": np.ascontiguousarray(z1_8[b * S:(b + 1) * S].T),
            "wq8": np.clip(np.ascontiguousarray(wq[sl].T) * SQK, -240, 240).astype(F8),
            "wk8": np.clip(np.ascontiguousarray(wk[sl].T) * SQK, -240, 240).astype(F8),
            "wv8": np.clip(np.ascontiguousarray(wv[sl].T) * SQK, -240, 240).astype(F8),
            "woT": (np.ascontiguousarray(wo[:, sl].T) / SQK).astype(BF16),
        })
    res1 = run_bass_kernel_spmd(nc1, in_maps, core_ids=list(range(NCORES)), trace=TRACE)
    LAST_RESULTS.append(res1)

    h1 = xf.copy()
    for c in range(NCORES):
        b = c // NGRP
        h1[b * S:(b + 1) * S] += res1.results[c]["h1p"].astype(np.float32)

    # ---- host: routing (exact fp32 semantics like the reference) ----
    z = _rmsnorm(h1, ln2_w)
    logits = (z.astype(np.float64) @ gate_w.T.astype(np.float64)).astype(np.float32)
    order = np.argsort(-logits, axis=-1, kind="stable")
    sel = order[:, :2]                               # top-2, ties -> lower index
    vals = np.take_along_axis(logits, sel, axis=-1).astype(np.float32)
    mx = vals.max(axis=-1, keepdims=True)
    ex = np.exp(vals - mx)
    rw = (ex / ex.sum(axis=-1, keepdims=True)).astype(np.float32)

    idx_lists = []
    for e in range(E):
        m = (sel == e)
        tok = np.nonzero(m.any(axis=-1))[0]
        wgt = np.where(m, rw, 0.0).sum(axis=-1)[tok]
        idx_lists.append((tok, wgt.astype(np.float32)))
    maxload = max(len(tok) for tok, _ in idx_lists)
    cap = C
    while cap < maxload:
        cap += 512
    nc2 = _get(f"l2_{cap}", _build_l2, cap)

    # ---- launch 2: expert-parallel FFN (fp8) ----
    zT = np.clip(np.ascontiguousarray(z.T), -240, 240).astype(F8)    # [H, T]
    in_maps2 = []
    for e in range(E):
        tok, wgt = idx_lists[e]
        zeT = np.zeros((H, cap), F8)
        zeT[:, :len(tok)] = zT[:, tok]
        web = np.zeros((cap,), np.float32)
        web[:len(tok)] = wgt / (SW3 * SW2)
        in_maps2.append({
            "zeT": zeT,
            "w1T": np.clip(np.ascontiguousarray(w1[e].T) * SW1, -240, 240).astype(F8),
            "w3T": np.clip(np.ascontiguousarray(w3[e].T) * SW3, -240, 240).astype(F8),
            "w2T": np.clip(np.ascontiguousarray(w2[e].T) * SW2, -240, 240).astype(F8),
            "web": np.broadcast_to(web, (128, cap)).copy(),
        })
    res2 = run_bass_kernel_spmd(nc2, in_maps2, core_ids=list(range(NCORES)), trace=TRACE)
    LAST_RESULTS.append(res2)

    out = h1.copy()
    for e in range(E):
        tok, _ = idx_lists[e]
        out[tok] += res2.results[e]["yT"][:, :len(tok)].T.astype(np.float32)

    return out.reshape(B, S, H).astype(np.float32)



# revision 34
# speedup vs baseline: 1.9139x; 1.0007x over previous
"""Trainium2 Bass kernel for a Mixtral decoder layer (attention + top-2 MoE).

Strategy (8 NeuronCores):
  Launch 1 (attention): 2D shard = (batch b in {0,1}) x (head-group g in {0..3},
    4 heads / 256 feature slice each). Each core computes q/k/v projections for
    its slice, transposed-scores flash-style attention (scores computed as
    s^T[tk, tq] so the softmax denominator folds into a ones-column of V), and
    a partial output projection. Host sums the 4 partials per batch.
  Host: residual add, rmsnorm, gating logits, exact top-2 routing, per-expert
    token gather (expert-parallel dispatch done in numpy - free).
  Launch 2 (MoE FFN): expert-parallel - core e owns expert e's w1/w3/w2 and
    processes its routed tokens (padded to capacity C) densely, pipelined over
    512-token blocks.
  Host: scatter-add expert outputs + residual. All matmuls bf16 with fp32 PSUM
    accumulation; softmax/normalization/routing in fp32.
"""
import os
import sys

import numpy as np
import ml_dtypes

for _p in ("/root/.axon_site", "/root/.axon_site/_ro/trn_rl_repo", "/opt/trn_rl_repo"):
    if os.path.isdir(_p) and _p not in sys.path:
        sys.path.append(_p)

import concourse.tile as tile
from concourse import bacc, mybir
from concourse.bass_utils import run_bass_kernel_spmd

BF16 = ml_dtypes.bfloat16
AF = mybir.ActivationFunctionType
ALU = mybir.AluOpType
DT = mybir.dt

H = 1024
S = 2048
B = 2
NH = 16
D = 64
E = 8
I = 2048
T = B * S
EPS = 1e-5

NCORES = 8
NGRP = 4              # head groups (cores per batch)
NHPC = NH // NGRP     # 4 heads per core
DS = NHPC * D         # 256-wide feature slice per core
TQC = 4               # tq chunks of 512
NTK = S // 128        # 16 tk tiles
NCI = H // 128        # 8 contraction chunks

C = 1088              # MoE expert token capacity (per-expert max on this data ~1087)
SW1, SW3, SW2 = 64.0, 16.0, 64.0   # fp8 weight scales (powers of 2, exact to undo)
F8 = ml_dtypes.float8_e4m3

_CACHE = {}
LAST_RESULTS = []     # BassKernelResults of the last kernel() call (for test harness)
TRACE = os.environ.get("KERNEL_TRACE", "0") == "1"

# tk tiles per attend-half whose softmax exp runs on DVE (custom fused op)
# instead of ACT, to balance the two engines
DVE_EXP_TKT = (5, 10, 15)


def _register_exp_ops():
    """Register two custom DVE ops computing exp via a degree-2 polynomial
    base and repeated squaring: op1 = (1 + w + w^2/2)^8 ~ e^{8w} with
    w = in*C0, op2 = x^4. Chained with C0 = scale/32 they give e^{scale*in}
    to ~1% relative accuracy on |scale*in| <= 4.2."""
    import concourse.dve_ops as dve_ops
    from concourse.dve_spec import Spec, Src0, C0, C1, One, sq, lower
    from concourse.dve_spec import _has_src1 as has_src1
    from concourse.dve_ops import DveOp, OPS, CUSTOM_DVE_SPECS, _SUB_OPCODE_FOR_NAME
    from concourse.dve_uop import DveOpSpec

    if "EXP_BASE_ANT" in CUSTOM_DVE_SPECS:
        return dve_ops.EXP_BASE_ANT, dve_ops.EXP_SQ2_ANT

    w = Src0 * C0
    p = One + w * (One + w * C1)

    def ref1(in0, in1, c0, c1, c2):
        ww = in0.astype(np.float32) * c0
        pp = 1.0 + ww * (1.0 + ww * c1)
        return (pp ** 8).astype(np.float32)

    def ref2(in0, in1, c0, c1, c2):
        x = in0.astype(np.float32)
        return (x * x) * (x * x)

    ops = []
    row = max(_SUB_OPCODE_FOR_NAME.values()) + 1
    for name, body, ref in (("EXP_BASE_ANT", sq(sq(sq(p))), ref1),
                            ("EXP_SQ2_ANT", sq(sq(Src0)), ref2)):
        spec = Spec(body=body, reference=ref)
        shas = {}
        for ver in ("v3", "v4"):
            s = DveOpSpec(name=name, opcode=row, uops=lower(spec, ver=ver),
                          rd1_en=has_src1(spec))
            shas[ver] = s.sha(ver)
        op = DveOp(name, spec, subdim=False, uops_sha=shas)
        _SUB_OPCODE_FOR_NAME[name] = row
        OPS.append(op)
        CUSTOM_DVE_SPECS[name] = spec
        setattr(dve_ops, name, op)
        ops.append(op)
        row += 1
    assert row <= 0x20
    return ops[0], ops[1]


def _capacity_chunks(cap):
    out, o = [], 0
    while o < cap:
        ln = min(512, cap - o)
        out.append((o, ln))
        o += ln
    return out


def _build_l1():
    """Attention, fp8-DoubleRow projections + flipped AV.

    Weights wq/wk/wv scaled by SQK=32 on host (fp8 range); q',k' = 32*true so
    scores = 1024*true, folded into the exp scale 2^-13. v' = 32*true; the
    AV output is 32*attn, normalized by the softmax denom (ones-column of v,
    unscaled), and 1/32 is folded into woT on host. AV is computed transposed:
    out[q_slice(128), d+1(65)] = pt[tk,q].T @ v[tk,65] so the denominator is a
    per-partition scalar and M=128 (full PE array)."""
    nc = bacc.Bacc("TRN2", target_bir_lowering=False, debug=False, num_devices=NCORES)
    xT8 = nc.dram_tensor("xT8", [H, S], DT.float8e4, kind="ExternalInput")
    wq8 = nc.dram_tensor("wq8", [H, DS], DT.float8e4, kind="ExternalInput")
    wk8 = nc.dram_tensor("wk8", [H, DS], DT.float8e4, kind="ExternalInput")
    wv8 = nc.dram_tensor("wv8", [H, DS], DT.float8e4, kind="ExternalInput")
    woT = nc.dram_tensor("woT", [DS, H], DT.bfloat16, kind="ExternalInput")
    h1p = nc.dram_tensor("h1p", [S, H], DT.bfloat16, kind="ExternalOutput")

    NPH = H // 256       # 4 H k-pairs for DoubleRow
    DR = mybir.MatmulPerfMode.DoubleRow
    EXPSC = 0.125 / (32.0 * 32.0)    # softmax 1/8 plus q,k weight scales
    EXP_BASE, EXP_SQ2 = _register_exp_ops()
    with tile.TileContext(nc) as tc:
        with tc.tile_pool(name="wpool", bufs=1) as wpool, \
             tc.tile_pool(name="qk", bufs=1) as qkpool, \
             tc.tile_pool(name="vp", bufs=1) as vpool, \
             tc.tile_pool(name="pt", bufs=4) as ptpool, \
             tc.tile_pool(name="ao", bufs=1) as aopool, \
             tc.tile_pool(name="at", bufs=3) as atpool, \
             tc.tile_pool(name="rc", bufs=4) as rcpool, \
             tc.tile_pool(name="st", bufs=2) as stpool, \
             tc.tile_pool(name="hout", bufs=4) as hpool, \
             tc.tile_pool(name="pp", bufs=2, space="PSUM") as pp, \
             tc.tile_pool(name="pav", bufs=1, space="PSUM") as pav, \
             tc.tile_pool(name="ppo", bufs=2, space="PSUM") as ppo:

            # ---- loads: x8/wq pairs first, wk next, wv later, wo last ----
            x8 = [[wpool.tile([128, 2, S // 2], DT.float8e4, name=f"x8_{p}_{hf}",
                              tag=f"x8_{p}_{hf}") for hf in range(2)]
                  for p in range(NPH)]
            # q/k weight tiles padded to DS+16 cols: a [.., 2, 128] slice of an
            # exactly-DS tile is fully contiguous, which walrus's LDW
            # optimization claims and then rejects for DoubleRow
            wq_t = [wpool.tile([128, 2, DS + 16], DT.float8e4, name=f"wqt{p}",
                               tag=f"wqt{p}") for p in range(NPH)]
            wk_t = [wpool.tile([128, 2, DS + 16], DT.float8e4, name=f"wkt{p}",
                               tag=f"wkt{p}") for p in range(NPH)]
            wv_t = [wpool.tile([128, 2, DS], DT.float8e4, name=f"wvt{p}",
                               tag=f"wvt{p}") for p in range(NPH)]
            xr = xT8.rearrange("(a two p) s -> a p two s", p=128, two=2)
            wqr = wq8.rearrange("(a two p) m -> a p two m", p=128, two=2)
            wkr = wk8.rearrange("(a two p) m -> a p two m", p=128, two=2)
            wvr = wv8.rearrange("(a two p) m -> a p two m", p=128, two=2)
            for p in range(NPH):
                nc.sync.dma_start(x8[p][0][:], xr[p][:, :, 0:S // 2])
                nc.sync.dma_start(wq_t[p][:, :, 0:DS], wqr[p])
                nc.sync.dma_start(wk_t[p][:, :, 0:DS], wkr[p])
            for p in range(NPH):
                nc.sync.dma_start(x8[p][1][:], xr[p][:, :, S // 2:S])
            for p in range(NPH):
                nc.sync.dma_start(wv_t[p][:], wvr[p])
            wo_sb = wpool.tile([128, DS // 128, H], DT.bfloat16)
            nc.sync.dma_start(wo_sb[:], woT.rearrange("(c p) m -> p c m", p=128))

            # q/k per head-pair [128, S] bf16 (partitions 0:64 = even head's d,
            # 64:128 = odd head's; scaled by 32); v for all heads in one
            # [128, tk-pair, 2, head, 72] fp8 tile (col 64 = ones)
            qts = [[qkpool.tile([128, S // 2], DT.bfloat16, name=f"q{p}{th}",
                                tag=f"q{p}{th}") for th in range(2)]
                   for p in range(NHPC // 2)]
            kts = [[qkpool.tile([128, S // 2], DT.bfloat16, name=f"k{p}{th}",
                                tag=f"k{p}{th}") for th in range(2)]
                   for p in range(NHPC // 2)]
            vall = vpool.tile([128, NTK // 2, 2, NHPC, 72], DT.float8e4)
            nc.vector.memset(vall[:, :, :, :, 64:65], 1.0)
            aoTs = [aopool.tile([128, DS // 128, S // 2], DT.bfloat16,
                                name=f"aoT{hf}", tag=f"aoT{hf}") for hf in range(2)]

            def make_qk(pair, th, wt, dst):
                # dst[pair][th][128, 1024] bf16; partitions 0:64 even head,
                # 64:128 odd head of the pair; values 32x
                ps = pp.tile([128, 1024], DT.float32, tag="pp", name="ps")
                for i in range(2):
                    for p in range(NPH):
                        nc.tensor.matmul(
                            ps[:, i * 512:(i + 1) * 512],
                            wt[p][:, :, pair * 128:(pair + 1) * 128],
                            x8[p][th][:, :, i * 512:(i + 1) * 512],
                            start=(p == 0), stop=(p == NPH - 1),
                            perf_mode=DR,
                        )
                nc.vector.tensor_copy(dst[pair][th][:, :], ps[:, :])

            def make_v():
                for tkt in range(NTK):
                    pv = ppo.tile([128, DS], DT.float32, tag="ppo", name="pv")
                    for p in range(NPH):
                        nc.tensor.matmul(
                            pv[:, 0:DS],
                            x8[p][tkt // 8][:, :, (tkt % 8) * 128:(tkt % 8 + 1) * 128],
                            wv_t[p][:, :, :],
                            start=(p == 0), stop=(p == NPH - 1),
                            perf_mode=DR,
                        )
                    nc.vector.tensor_copy(
                        vall[:, tkt // 2, tkt % 2, :, 0:64],
                        pv[:, 0:DS].rearrange("p (h d) -> p h d", d=64))

            def av_mms(h, av, ptp, j):
                for qs in range(8):
                    nc.tensor.matmul(
                        av[:, qs, 0:65],
                        ptp[:, :, qs * 128:(qs + 1) * 128],
                        vall[:, j, :, h, 0:65],
                        start=(j == 0), stop=(j == NTK // 2 - 1),
                        perf_mode=DR,
                    )

            def attend(h, half):
                # one tq half (1024 queries, 8 slices of 128) of head h;
                # av[q_slice, qs, 0:64] = unnormalized attn (32x), [.., 64] = denom
                qt, kt = qts[h // 2][half], kts[h // 2]
                ro = (h % 2) * 64
                av = pav.tile([128, 8, 128], DT.float32, tag="pav", name="av")
                pending = None
                ptp = None
                for tkt in range(NTK):
                    sc = pp.tile([128, 1024], DT.float32, tag="pp", name="sc")
                    for i in range(2):
                        nc.tensor.matmul(
                            sc[:, i * 512:(i + 1) * 512],
                            kt[tkt // 8][ro:ro + 64, (tkt % 8) * 128:(tkt % 8 + 1) * 128],
                            qt[ro:ro + 64, i * 512:(i + 1) * 512],
                            start=True, stop=True,
                        )
                    if tkt % 2 == 0:
                        ptp = ptpool.tile([128, 2, 1024], DT.float8e4, tag="pt")
                    if tkt in DVE_EXP_TKT:
                        st = stpool.tile([128, 1024], DT.bfloat16, tag="st")
                        nc.vector._custom_dve(EXP_BASE, out=st[:], in0=sc[:],
                                              s0=EXPSC / 32.0, s1=0.5)
                        nc.vector._custom_dve(EXP_SQ2, out=ptp[:, tkt % 2, :],
                                              in0=st[:])
                    else:
                        nc.scalar.activation(ptp[:, tkt % 2, :], sc[:], AF.Exp,
                                             scale=EXPSC)
                    if tkt % 2 == 1:
                        if pending is not None:
                            av_mms(h, av, *pending)
                        pending = (ptp, tkt // 2)
                av_mms(h, av, *pending)
                # two heads of a pair share an at2 tile: cols (h%2)*64..+64;
                # transposed into aoTs after the odd head (see tp_pair)
                if h % 2 == 0:
                    at2[(h // 2, half)] = atpool.tile([128, 8, 128], DT.bfloat16,
                                                      tag="at", name="at2")
                at = at2[(h // 2, half)]
                roff = (h % 2) * 64
                rec = rcpool.tile([128, 8], DT.float32, tag="rc")
                nc.vector.reciprocal(rec[:, :], av[:, :, 64])
                nc.vector.tensor_tensor(
                    at[:, :, roff:roff + 64], av[:, :, 0:64],
                    rec[:, :, None].to_broadcast([128, 8, 64]), ALU.mult)

            def tp_pair(pair, half):
                at = at2.pop((pair, half))
                for qs in range(8):
                    nc.sync.dma_start_transpose(
                        aoTs[half][:, pair, qs * 128:(qs + 1) * 128],
                        at[:, qs, :])

            def oproj(half, ts):
                # h1p[tq, :] partial for the given tq tiles of this half
                for t in ts:
                    tkt = half * 8 + t
                    ht = hpool.tile([128, H], DT.bfloat16, tag="ht")
                    for jc in range(2):
                        po = ppo.tile([128, 512], DT.float32, tag="ppo", name="po")
                        for oc in range(2):
                            nc.tensor.matmul(
                                po[:, :],
                                aoTs[half][:, oc, t * 128:(t + 1) * 128],
                                wo_sb[:, oc, jc * 512:(jc + 1) * 512],
                                start=(oc == 0), stop=(oc == 1),
                            )
                        nc.vector.tensor_copy(ht[:, jc * 512:(jc + 1) * 512], po[:])
                    nc.sync.dma_start(h1p[tkt * 128:(tkt + 1) * 128, :], ht[:])

            at2 = {}
            make_qk(0, 0, wq_t, qts)
            make_qk(0, 0, wk_t, kts)
            make_qk(0, 1, wq_t, qts)
            make_qk(0, 1, wk_t, kts)
            make_qk(1, 0, wq_t, qts)
            make_qk(1, 0, wk_t, kts)
            make_qk(1, 1, wq_t, qts)
            make_qk(1, 1, wk_t, kts)
            make_v()
            attend(0, 0)
            attend(1, 0)
            tp_pair(0, 0)
            attend(2, 0)
            attend(3, 0)
            tp_pair(1, 0)
            # oproj(0) interleaved with half-1 attends so PE never displaces
            # the ACT-critical scores for long
            attend(0, 1)
            oproj(0, range(0, 3))
            attend(1, 1)
            oproj(0, range(3, 6))
            tp_pair(0, 1)
            attend(2, 1)
            oproj(0, range(6, 8))
            attend(3, 1)
            tp_pair(1, 1)
            oproj(1, range(8))

    nc.compile()
    nc.finalize()
    return nc


def _build_l2(cap):
    """Expert-parallel MoE FFN, fp8e4m3 + DoubleRow (2 K-tiles/instruction).

    Scales: w1 *= SW1 (silu scale=1/SW1 undoes exactly), w3 *= SW3,
    w2 *= SW2; web divided by SW3*SW2 on host. hh = silu_true * SW3*g_true
    stays well under fp8e4 max 240 for SW3=16."""
    nc = bacc.Bacc("TRN2", target_bir_lowering=False, debug=False, num_devices=NCORES)
    zeT = nc.dram_tensor("zeT", [H, cap], DT.float8e4, kind="ExternalInput")
    w1T = nc.dram_tensor("w1T", [H, I], DT.float8e4, kind="ExternalInput")
    w3T = nc.dram_tensor("w3T", [H, I], DT.float8e4, kind="ExternalInput")
    w2T = nc.dram_tensor("w2T", [I, H], DT.float8e4, kind="ExternalInput")
    web = nc.dram_tensor("web", [128, cap], DT.float32, kind="ExternalInput")
    yT = nc.dram_tensor("yT", [H, cap], DT.bfloat16, kind="ExternalOutput")

    cch = _capacity_chunks(cap)
    NIC = I // 128
    NPH = H // 256        # 4 K-pairs over H
    NPI = I // 256        # 8 K-pairs over I
    DR = mybir.MatmulPerfMode.DoubleRow
    with tile.TileContext(nc) as tc:
        with tc.tile_pool(name="wpool", bufs=1) as wpool, \
             tc.tile_pool(name="hh", bufs=1) as hhpool, \
             tc.tile_pool(name="hs", bufs=3) as hspool, \
             tc.tile_pool(name="yt", bufs=3) as ytpool, \
             tc.tile_pool(name="pg", bufs=4, space="PSUM") as pg, \
             tc.tile_pool(name="py", bufs=4, space="PSUM") as py:

            # K-pair tiles; w1/w3 split in I-halves, loads interleaved per
            # pair so the h-phase streams behind the DMAs
            zps = [wpool.tile([128, 2, cap], DT.float8e4, name=f"zp{p}",
                              tag=f"zp{p}") for p in range(NPH)]
            w1ps = [[wpool.tile([128, 2, I // 2], DT.float8e4, name=f"w1p{p}_{b}",
                                tag=f"w1p{p}_{b}") for b in range(2)]
                    for p in range(NPH)]
            w3ps = [[wpool.tile([128, 2, I // 2], DT.float8e4, name=f"w3p{p}_{b}",
                                tag=f"w3p{p}_{b}") for b in range(2)]
                    for p in range(NPH)]
            zr = zeT.rearrange("(a two p) m -> a p two m", p=128, two=2)
            w1r = w1T.rearrange("(a two p) m -> a p two m", p=128, two=2)
            w3r = w3T.rearrange("(a two p) m -> a p two m", p=128, two=2)
            for p in range(NPH):
                nc.sync.dma_start(zps[p][:], zr[p])
                nc.sync.dma_start(w1ps[p][0][:], w1r[p][:, :, 0:I // 2])
                nc.sync.dma_start(w3ps[p][0][:], w3r[p][:, :, 0:I // 2])
            for p in range(NPH):
                nc.sync.dma_start(w1ps[p][1][:], w1r[p][:, :, I // 2:I])
                nc.sync.dma_start(w3ps[p][1][:], w3r[p][:, :, I // 2:I])
            web_sb = wpool.tile([128, cap], DT.float32)
            nc.sync.dma_start(web_sb[:], web[:, :])

            # hh as K-pair tiles over I for the DoubleRow y-phase
            hhp = [hhpool.tile([128, 2, cap], DT.float8e4, name=f"hhp{p}",
                               tag=f"hhp{p}") for p in range(NPI)]
            w2_holder = []

            for ic in range(NIC):
                b, bo = ic // 8, (ic % 8) * 128
                hs = hspool.tile([128, cap], DT.float8e4, tag="hs", name="hs")
                for j, (o, ln) in enumerate(cch):
                    hp = pg.tile([128, 512], DT.float32, tag="pg", name="hp")
                    for p in range(NPH):
                        nc.tensor.matmul(
                            hp[:, 0:ln],
                            w1ps[p][b][:, :, bo:bo + 128],
                            zps[p][:, :, o:o + ln],
                            start=(p == 0), stop=(p == NPH - 1),
                            perf_mode=DR,
                        )
                    nc.scalar.activation(hs[:, o:o + ln], hp[:, 0:ln], AF.Silu,
                                         scale=1.0 / SW1)
                    gp = pg.tile([128, 512], DT.float32, tag="pg", name="gp")
                    for p in range(NPH):
                        nc.tensor.matmul(
                            gp[:, 0:ln],
                            w3ps[p][b][:, :, bo:bo + 128],
                            zps[p][:, :, o:o + ln],
                            start=(p == 0), stop=(p == NPH - 1),
                            perf_mode=DR,
                        )
                    nc.vector.tensor_tensor(
                        hhp[ic // 2][:, ic % 2, o:o + ln],
                        gp[:, 0:ln], hs[:, o:o + ln], ALU.mult)
                if ic == 0:
                    # emit w2 load after the first h-block for DMA priority
                    w2ps = [wpool.tile([128, 2, H], DT.float8e4, name=f"w2p{p}",
                                       tag=f"w2p{p}") for p in range(NPI)]
                    w2r = w2T.rearrange("(a two p) m -> a p two m", p=128, two=2)
                    for p in range(NPI):
                        nc.sync.dma_start(w2ps[p][:], w2r[p])
                    w2_holder.append(w2ps)

            w2ps = w2_holder[0]
            for hc in range(NCI):
                yt = ytpool.tile([128, cap], DT.bfloat16, tag="yt", name="yt")
                for j, (o, ln) in enumerate(cch):
                    yp = py.tile([128, 512], DT.float32, tag="py", name="yp")
                    for p in range(NPI):
                        nc.tensor.matmul(
                            yp[:, 0:ln],
                            w2ps[p][:, :, hc * 128:(hc + 1) * 128],
                            hhp[p][:, :, o:o + ln],
                            start=(p == 0), stop=(p == NPI - 1),
                            perf_mode=DR,
                        )
                    nc.vector.tensor_tensor(
                        yt[:, o:o + ln], yp[:, 0:ln], web_sb[:, o:o + ln], ALU.mult)
                nc.sync.dma_start(yT[hc * 128:(hc + 1) * 128, :], yt[:])

    nc.compile()
    nc.finalize()
    return nc


def _get(name, builder, *args):
    if name not in _CACHE:
        _CACHE[name] = builder(*args)
    return _CACHE[name]


def _rmsnorm(x, w):
    xf = x.astype(np.float32)
    rms = 1.0 / np.sqrt((xf * xf).mean(axis=-1, keepdims=True) + EPS)
    return (xf * rms) * w.astype(np.float32)


def kernel(x, ln1_w, ln2_w, wq, wk, wv, wo, gate_w, w1, w2, w3):
    global LAST_RESULTS
    LAST_RESULTS = []
    x = np.asarray(x, np.float32)
    wq, wk, wv, wo = (np.asarray(a, np.float32) for a in (wq, wk, wv, wo))
    gate_w = np.asarray(gate_w, np.float32)
    w1, w2, w3 = (np.asarray(a, np.float32) for a in (w1, w2, w3))
    ln1_w = np.asarray(ln1_w, np.float32)
    ln2_w = np.asarray(ln2_w, np.float32)

    xf = x.reshape(T, H)
    z1 = _rmsnorm(xf, ln1_w)
    # ---- launch 1: attention (fp8 projections, bf16 scores/O-proj) ----
    SQK = 32.0
    nc1 = _get("l1", _build_l1)
    z1_8 = np.clip(z1, -240, 240).astype(F8)
    in_maps = []
    for c in range(NCORES):
        b, g = divmod(c, NGRP)
        sl = slice(g * DS, (g + 1) * DS)
        in_maps.append({
            "xT8": np.ascontiguousarray(z1_8[b * S:(b + 1) * S].T),
            "wq8": np.clip(np.ascontiguousarray(wq[sl].T) * SQK, -240, 240).astype(F8),
            "wk8": np.clip(np.ascontiguousarray(wk[sl].T) * SQK, -240, 240).astype(F8),
            "wv8": np.clip(np.ascontiguousarray(wv[sl].T) * SQK, -240, 240).astype(F8),
            "woT": (np.ascontiguousarray(wo[:, sl].T) / SQK).astype(BF16),
        })
    res1 = run_bass_kernel_spmd(nc1, in_maps, core_ids=list(range(NCORES)), trace=TRACE)
    LAST_RESULTS.append(res1)

    h1 = xf.copy()
    for c in range(NCORES):
        b = c // NGRP
        h1[b * S:(b + 1) * S] += res1.results[c]["h1p"].astype(np.float32)

    # ---- host: routing (exact fp32 semantics like the reference) ----
    z = _rmsnorm(h1, ln2_w)
    logits = (z.astype(np.float64) @ gate_w.T.astype(np.float64)).astype(np.float32)
    order = np.argsort(-logits, axis=-1, kind="stable")
    sel = order[:, :2]                               # top-2, ties -> lower index
    vals = np.take_along_axis(logits, sel, axis=-1).astype(np.float32)
    mx = vals.max(axis=-1, keepdims=True)
    ex = np.exp(vals - mx)
    rw = (ex / ex.sum(axis=-1, keepdims=True)).astype(np.float32)

    idx_lists = []
    for e in range(E):
        m = (sel == e)
        tok = np.nonzero(m.any(axis=-1))[0]
        wgt = np.where(m, rw, 0.0).sum(axis=-1)[tok]
        idx_lists.append((tok, wgt.astype(np.float32)))
    maxload = max(len(tok) for tok, _ in idx_lists)
    cap = C
    while cap < maxload:
        cap += 512
    nc2 = _get(f"l2_{cap}", _build_l2, cap)

    # ---- launch 2: expert-parallel FFN (fp8) ----
    zT = np.clip(np.ascontiguousarray(z.T), -240, 240).astype(F8)    # [H, T]
    in_maps2 = []
    for e in range(E):
        tok, wgt = idx_lists[e]
        zeT = np.zeros((H, cap), F8)
        zeT[:, :len(tok)] = zT[:, tok]
        web = np.zeros((cap,), np.float32)
        web[:len(tok)] = wgt / (SW3 * SW2)
        in_maps2.append({
            "zeT": zeT,
            "w1T": np.clip(np.ascontiguousarray(w1[e].T) * SW1, -240, 240).astype(F8),
            "w3T": np.clip(np.ascontiguousarray(w3[e].T) * SW3, -240, 240).astype(F8),
            "w2T": np.clip(np.ascontiguousarray(w2[e].T) * SW2, -240, 240).astype(F8),
            "web": np.broadcast_to(web, (128, cap)).copy(),
        })
    res2 = run_bass_kernel_spmd(nc2, in_maps2, core_ids=list(range(NCORES)), trace=TRACE)
    LAST_RESULTS.append(res2)

    out = h1.copy()
    for e in range(E):
        tok, _ = idx_lists[e]
        out[tok] += res2.results[e]["yT"][:, :len(tok)].T.astype(np.float32)

    return out.reshape(B, S, H).astype(np.float32)



# revision 35
# speedup vs baseline: 1.9343x; 1.0107x over previous
"""Trainium2 Bass kernel for a Mixtral decoder layer (attention + top-2 MoE).

Strategy (8 NeuronCores):
  Launch 1 (attention): 2D shard = (batch b in {0,1}) x (head-group g in {0..3},
    4 heads / 256 feature slice each). Each core computes q/k/v projections for
    its slice, transposed-scores flash-style attention (scores computed as
    s^T[tk, tq] so the softmax denominator folds into a ones-column of V), and
    a partial output projection. Host sums the 4 partials per batch.
  Host: residual add, rmsnorm, gating logits, exact top-2 routing, per-expert
    token gather (expert-parallel dispatch done in numpy - free).
  Launch 2 (MoE FFN): expert-parallel - core e owns expert e's w1/w3/w2 and
    processes its routed tokens (padded to capacity C) densely, pipelined over
    512-token blocks.
  Host: scatter-add expert outputs + residual. All matmuls bf16 with fp32 PSUM
    accumulation; softmax/normalization/routing in fp32.
"""
import os
import sys

import numpy as np
import ml_dtypes

for _p in ("/root/.axon_site", "/root/.axon_site/_ro/trn_rl_repo", "/opt/trn_rl_repo"):
    if os.path.isdir(_p) and _p not in sys.path:
        sys.path.append(_p)

import concourse.tile as tile
from concourse import bacc, mybir
from concourse.bass_utils import run_bass_kernel_spmd

BF16 = ml_dtypes.bfloat16
AF = mybir.ActivationFunctionType
ALU = mybir.AluOpType
DT = mybir.dt

H = 1024
S = 2048
B = 2
NH = 16
D = 64
E = 8
I = 2048
T = B * S
EPS = 1e-5

NCORES = 8
NGRP = 4              # head groups (cores per batch)
NHPC = NH // NGRP     # 4 heads per core
DS = NHPC * D         # 256-wide feature slice per core
TQC = 4               # tq chunks of 512
NTK = S // 128        # 16 tk tiles
NCI = H // 128        # 8 contraction chunks

C = 1088              # MoE expert token capacity (per-expert max on this data ~1087)
SW1, SW3, SW2 = 64.0, 16.0, 64.0   # fp8 weight scales (powers of 2, exact to undo)
F8 = ml_dtypes.float8_e4m3

_CACHE = {}
LAST_RESULTS = []     # BassKernelResults of the last kernel() call (for test harness)
TRACE = os.environ.get("KERNEL_TRACE", "0") == "1"

# tk tiles per attend-half whose softmax exp runs on DVE (custom fused op)
# instead of ACT, to balance the two engines
DVE_EXP_TKT = (5, 10, 15)


def _register_exp_ops():
    """Register two custom DVE ops computing exp via a degree-2 polynomial
    base and repeated squaring: op1 = (1 + w + w^2/2)^8 ~ e^{8w} with
    w = in*C0, op2 = x^4. Chained with C0 = scale/32 they give e^{scale*in}
    to ~1% relative accuracy on |scale*in| <= 4.2."""
    import concourse.dve_ops as dve_ops
    from concourse.dve_spec import Spec, Src0, C0, C1, One, sq, lower
    from concourse.dve_spec import _has_src1 as has_src1
    from concourse.dve_ops import DveOp, OPS, CUSTOM_DVE_SPECS, _SUB_OPCODE_FOR_NAME
    from concourse.dve_uop import DveOpSpec

    if "EXP_BASE_ANT" in CUSTOM_DVE_SPECS:
        return dve_ops.EXP_BASE_ANT, dve_ops.EXP_SQ2_ANT

    w = Src0 * C0
    p = One + w * (One + w * C1)

    def ref1(in0, in1, c0, c1, c2):
        ww = in0.astype(np.float32) * c0
        pp = 1.0 + ww * (1.0 + ww * c1)
        return (pp ** 8).astype(np.float32)

    def ref2(in0, in1, c0, c1, c2):
        x = in0.astype(np.float32)
        return (x * x) * (x * x)

    ops = []
    row = max(_SUB_OPCODE_FOR_NAME.values()) + 1
    for name, body, ref in (("EXP_BASE_ANT", sq(sq(sq(p))), ref1),
                            ("EXP_SQ2_ANT", sq(sq(Src0)), ref2)):
        spec = Spec(body=body, reference=ref)
        shas = {}
        for ver in ("v3", "v4"):
            s = DveOpSpec(name=name, opcode=row, uops=lower(spec, ver=ver),
                          rd1_en=has_src1(spec))
            shas[ver] = s.sha(ver)
        op = DveOp(name, spec, subdim=False, uops_sha=shas)
        _SUB_OPCODE_FOR_NAME[name] = row
        OPS.append(op)
        CUSTOM_DVE_SPECS[name] = spec
        setattr(dve_ops, name, op)
        ops.append(op)
        row += 1
    assert row <= 0x20
    return ops[0], ops[1]


def _capacity_chunks(cap):
    out, o = [], 0
    while o < cap:
        ln = min(512, cap - o)
        out.append((o, ln))
        o += ln
    return out


def _build_l1():
    """Attention, fp8-DoubleRow projections + flipped AV.

    Weights wq/wk/wv scaled by SQK=32 on host (fp8 range); q',k' = 32*true so
    scores = 1024*true, folded into the exp scale 2^-13. v' = 32*true; the
    AV output is 32*attn, normalized by the softmax denom (ones-column of v,
    unscaled), and 1/32 is folded into woT on host. AV is computed transposed:
    out[q_slice(128), d+1(65)] = pt[tk,q].T @ v[tk,65] so the denominator is a
    per-partition scalar and M=128 (full PE array)."""
    nc = bacc.Bacc("TRN2", target_bir_lowering=False, debug=False, num_devices=NCORES)
    xT8 = nc.dram_tensor("xT8", [H, S], DT.float8e4, kind="ExternalInput")
    wqkv8 = nc.dram_tensor("wqkv8", [H, 3 * DS], DT.float8e4, kind="ExternalInput")
    woT = nc.dram_tensor("woT", [DS, H], DT.bfloat16, kind="ExternalInput")
    h1p = nc.dram_tensor("h1p", [S, H], DT.bfloat16, kind="ExternalOutput")

    NPH = H // 256       # 4 H k-pairs for DoubleRow
    DR = mybir.MatmulPerfMode.DoubleRow
    EXPSC = 0.125 / (32.0 * 32.0)    # softmax 1/8 plus q,k weight scales
    EXP_BASE, EXP_SQ2 = _register_exp_ops()
    with tile.TileContext(nc) as tc:
        with tc.tile_pool(name="wpool", bufs=1) as wpool, \
             tc.tile_pool(name="qk", bufs=1) as qkpool, \
             tc.tile_pool(name="vp", bufs=1) as vpool, \
             tc.tile_pool(name="pt", bufs=4) as ptpool, \
             tc.tile_pool(name="ao", bufs=1) as aopool, \
             tc.tile_pool(name="at", bufs=3) as atpool, \
             tc.tile_pool(name="rc", bufs=4) as rcpool, \
             tc.tile_pool(name="st", bufs=2) as stpool, \
             tc.tile_pool(name="hout", bufs=4) as hpool, \
             tc.tile_pool(name="pp", bufs=2, space="PSUM") as pp, \
             tc.tile_pool(name="pav", bufs=1, space="PSUM") as pav, \
             tc.tile_pool(name="ppo", bufs=2, space="PSUM") as ppo:

            # ---- loads: 4 big DMAs (wqkv, x half 0, x half 1, wo) ----
            wqkv_t = wpool.tile([128, NPH, 2, 3 * DS], DT.float8e4)
            nc.sync.dma_start(
                wqkv_t[:], wqkv8.rearrange("(a two p) m -> p a two m", p=128, two=2))
            x8h = [wpool.tile([128, NPH, 2, S // 2], DT.float8e4, name=f"x8h{hf}",
                              tag=f"x8h{hf}") for hf in range(2)]
            xr = xT8.rearrange("(a two p) s -> p a two s", p=128, two=2)
            nc.sync.dma_start(x8h[0][:], xr[:, :, :, 0:S // 2])
            nc.sync.dma_start(x8h[1][:], xr[:, :, :, S // 2:S])
            wo_sb = wpool.tile([128, DS // 128, H], DT.bfloat16)
            nc.sync.dma_start(wo_sb[:], woT.rearrange("(c p) m -> p c m", p=128))

            # q/k per head-pair [128, S] bf16 (partitions 0:64 = even head's d,
            # 64:128 = odd head's; scaled by 32); v for all heads in one
            # [128, tk-pair, 2, head, 72] fp8 tile (col 64 = ones)
            qts = [[qkpool.tile([128, S // 2], DT.bfloat16, name=f"q{p}{th}",
                                tag=f"q{p}{th}") for th in range(2)]
                   for p in range(NHPC // 2)]
            kts = [[qkpool.tile([128, S // 2], DT.bfloat16, name=f"k{p}{th}",
                                tag=f"k{p}{th}") for th in range(2)]
                   for p in range(NHPC // 2)]
            vall = vpool.tile([128, NTK // 2, 2, NHPC, 72], DT.float8e4)
            nc.vector.memset(vall[:, :, :, :, 64:65], 1.0)
            aoTs = [aopool.tile([128, DS // 128, S // 2], DT.bfloat16,
                                name=f"aoT{hf}", tag=f"aoT{hf}") for hf in range(2)]

            def make_qk(pair, th, woff, dst, on_act=False):
                # dst[pair][th][128, 1024] bf16; partitions 0:64 even head,
                # 64:128 odd head of the pair; values 32x. woff: 0=q, DS=k.
                ps = pp.tile([128, 1024], DT.float32, tag="pp", name="ps")
                for i in range(2):
                    for p in range(NPH):
                        nc.tensor.matmul(
                            ps[:, i * 512:(i + 1) * 512],
                            wqkv_t[:, p, :, woff + pair * 128:woff + (pair + 1) * 128],
                            x8h[th][:, p, :, i * 512:(i + 1) * 512],
                            start=(p == 0), stop=(p == NPH - 1),
                            perf_mode=DR,
                        )
                if on_act:
                    nc.scalar.activation(dst[pair][th][:, :], ps[:, :], AF.Copy)
                else:
                    nc.vector.tensor_copy(dst[pair][th][:, :], ps[:, :])

            def make_v():
                for tkt in range(NTK):
                    pv = ppo.tile([128, DS], DT.float32, tag="ppo", name="pv")
                    for p in range(NPH):
                        nc.tensor.matmul(
                            pv[:, 0:DS],
                            x8h[tkt // 8][:, p, :, (tkt % 8) * 128:(tkt % 8 + 1) * 128],
                            wqkv_t[:, p, :, 2 * DS:3 * DS],
                            start=(p == 0), stop=(p == NPH - 1),
                            perf_mode=DR,
                        )
                    nc.vector.tensor_copy(
                        vall[:, tkt // 2, tkt % 2, :, 0:64],
                        pv[:, 0:DS].rearrange("p (h d) -> p h d", d=64))

            def av_mms(h, av, ptp, j):
                for qs in range(8):
                    nc.tensor.matmul(
                        av[:, qs, 0:65],
                        ptp[:, :, qs * 128:(qs + 1) * 128],
                        vall[:, j, :, h, 0:65],
                        start=(j == 0), stop=(j == NTK // 2 - 1),
                        perf_mode=DR,
                    )

            def attend(h, half):
                # one tq half (1024 queries, 8 slices of 128) of head h;
                # av[q_slice, qs, 0:64] = unnormalized attn (32x), [.., 64] = denom
                qt, kt = qts[h // 2][half], kts[h // 2]
                ro = (h % 2) * 64
                av = pav.tile([128, 8, 128], DT.float32, tag="pav", name="av")
                pending = None
                ptp = None
                for tkt in range(NTK):
                    sc = pp.tile([128, 1024], DT.float32, tag="pp", name="sc")
                    for i in range(2):
                        nc.tensor.matmul(
                            sc[:, i * 512:(i + 1) * 512],
                            kt[tkt // 8][ro:ro + 64, (tkt % 8) * 128:(tkt % 8 + 1) * 128],
                            qt[ro:ro + 64, i * 512:(i + 1) * 512],
                            start=True, stop=True,
                        )
                    if tkt % 2 == 0:
                        ptp = ptpool.tile([128, 2, 1024], DT.float8e4, tag="pt")
                    if tkt in DVE_EXP_TKT:
                        st = stpool.tile([128, 1024], DT.bfloat16, tag="st")
                        nc.vector._custom_dve(EXP_BASE, out=st[:], in0=sc[:],
                                              s0=EXPSC / 32.0, s1=0.5)
                        nc.vector._custom_dve(EXP_SQ2, out=ptp[:, tkt % 2, :],
                                              in0=st[:])
                    else:
                        nc.scalar.activation(ptp[:, tkt % 2, :], sc[:], AF.Exp,
                                             scale=EXPSC)
                    if tkt % 2 == 1:
                        if pending is not None:
                            av_mms(h, av, *pending)
                        pending = (ptp, tkt // 2)
                av_mms(h, av, *pending)
                # two heads of a pair share an at2 tile: cols (h%2)*64..+64;
                # transposed into aoTs after the odd head (see tp_pair)
                if h % 2 == 0:
                    at2[(h // 2, half)] = atpool.tile([128, 8, 128], DT.bfloat16,
                                                      tag="at", name="at2")
                at = at2[(h // 2, half)]
                roff = (h % 2) * 64
                rec = rcpool.tile([128, 8], DT.float32, tag="rc")
                nc.vector.reciprocal(rec[:, :], av[:, :, 64])
                nc.vector.tensor_tensor(
                    at[:, :, roff:roff + 64], av[:, :, 0:64],
                    rec[:, :, None].to_broadcast([128, 8, 64]), ALU.mult)

            def tp_pair(pair, half):
                # one blocked transpose: out[j, b, q] = at[q, b, j]
                at = at2.pop((pair, half))
                nc.sync.dma_start_transpose(
                    aoTs[half][:, pair, 0:1024].rearrange("p (b q) -> p b q", q=128),
                    at.rearrange("p a b -> p (a b)"))

            def oproj(half, ts):
                # h1p[tq, :] partial for the given tq tiles of this half
                for t in ts:
                    tkt = half * 8 + t
                    ht = hpool.tile([128, H], DT.bfloat16, tag="ht")
                    for jc in range(2):
                        po = ppo.tile([128, 512], DT.float32, tag="ppo", name="po")
                        for oc in range(2):
                            nc.tensor.matmul(
                                po[:, :],
                                aoTs[half][:, oc, t * 128:(t + 1) * 128],
                                wo_sb[:, oc, jc * 512:(jc + 1) * 512],
                                start=(oc == 0), stop=(oc == 1),
                            )
                        nc.vector.tensor_copy(ht[:, jc * 512:(jc + 1) * 512], po[:])
                    nc.sync.dma_start(h1p[tkt * 128:(tkt + 1) * 128, :], ht[:])

            at2 = {}
            make_qk(0, 0, 0, qts)
            make_qk(0, 0, DS, kts, on_act=True)   # ACT: parallel startup
            make_qk(0, 1, 0, qts)
            make_qk(0, 1, DS, kts, on_act=True)
            make_qk(1, 0, 0, qts)
            make_qk(1, 0, DS, kts)
            make_qk(1, 1, 0, qts)
            make_qk(1, 1, DS, kts)
            make_v()
            attend(0, 0)
            attend(1, 0)
            tp_pair(0, 0)
            attend(2, 0)
            attend(3, 0)
            tp_pair(1, 0)
            # oproj(0) interleaved with half-1 attends so PE never displaces
            # the ACT-critical scores for long
            attend(0, 1)
            oproj(0, range(0, 3))
            attend(1, 1)
            oproj(0, range(3, 6))
            tp_pair(0, 1)
            attend(2, 1)
            oproj(0, range(6, 8))
            attend(3, 1)
            tp_pair(1, 1)
            oproj(1, range(8))

    nc.compile()
    nc.finalize()
    return nc


def _build_l2(cap):
    """Expert-parallel MoE FFN, fp8e4m3 + DoubleRow (2 K-tiles/instruction).

    Scales: w1 *= SW1 (silu scale=1/SW1 undoes exactly), w3 *= SW3,
    w2 *= SW2; web divided by SW3*SW2 on host. hh = silu_true * SW3*g_true
    stays well under fp8e4 max 240 for SW3=16."""
    nc = bacc.Bacc("TRN2", target_bir_lowering=False, debug=False, num_devices=NCORES)
    zeT = nc.dram_tensor("zeT", [H, cap], DT.float8e4, kind="ExternalInput")
    w1T = nc.dram_tensor("w1T", [H, I], DT.float8e4, kind="ExternalInput")
    w3T = nc.dram_tensor("w3T", [H, I], DT.float8e4, kind="ExternalInput")
    w2T = nc.dram_tensor("w2T", [I, H], DT.float8e4, kind="ExternalInput")
    web = nc.dram_tensor("web", [128, cap], DT.float32, kind="ExternalInput")
    yT = nc.dram_tensor("yT", [H, cap], DT.bfloat16, kind="ExternalOutput")

    cch = _capacity_chunks(cap)
    NIC = I // 128
    NPH = H // 256        # 4 K-pairs over H
    NPI = I // 256        # 8 K-pairs over I
    DR = mybir.MatmulPerfMode.DoubleRow
    with tile.TileContext(nc) as tc:
        with tc.tile_pool(name="wpool", bufs=1) as wpool, \
             tc.tile_pool(name="hh", bufs=1) as hhpool, \
             tc.tile_pool(name="hs", bufs=3) as hspool, \
             tc.tile_pool(name="yt", bufs=3) as ytpool, \
             tc.tile_pool(name="pg", bufs=4, space="PSUM") as pg, \
             tc.tile_pool(name="py", bufs=4, space="PSUM") as py:

            # K-pair tiles; w1/w3 split in I-halves, loads interleaved per
            # pair so the h-phase streams behind the DMAs
            zps = [wpool.tile([128, 2, cap], DT.float8e4, name=f"zp{p}",
                              tag=f"zp{p}") for p in range(NPH)]
            w1ps = [[wpool.tile([128, 2, I // 2], DT.float8e4, name=f"w1p{p}_{b}",
                                tag=f"w1p{p}_{b}") for b in range(2)]
                    for p in range(NPH)]
            w3ps = [[wpool.tile([128, 2, I // 2], DT.float8e4, name=f"w3p{p}_{b}",
                                tag=f"w3p{p}_{b}") for b in range(2)]
                    for p in range(NPH)]
            zr = zeT.rearrange("(a two p) m -> a p two m", p=128, two=2)
            w1r = w1T.rearrange("(a two p) m -> a p two m", p=128, two=2)
            w3r = w3T.rearrange("(a two p) m -> a p two m", p=128, two=2)
            for p in range(NPH):
                nc.sync.dma_start(zps[p][:], zr[p])
                nc.sync.dma_start(w1ps[p][0][:], w1r[p][:, :, 0:I // 2])
                nc.sync.dma_start(w3ps[p][0][:], w3r[p][:, :, 0:I // 2])
            for p in range(NPH):
                nc.sync.dma_start(w1ps[p][1][:], w1r[p][:, :, I // 2:I])
                nc.sync.dma_start(w3ps[p][1][:], w3r[p][:, :, I // 2:I])
            web_sb = wpool.tile([128, cap], DT.float32)
            nc.sync.dma_start(web_sb[:], web[:, :])

            # hh as K-pair tiles over I for the DoubleRow y-phase
            hhp = [hhpool.tile([128, 2, cap], DT.float8e4, name=f"hhp{p}",
                               tag=f"hhp{p}") for p in range(NPI)]
            w2_holder = []

            for ic in range(NIC):
                b, bo = ic // 8, (ic % 8) * 128
                hs = hspool.tile([128, cap], DT.float8e4, tag="hs", name="hs")
                for j, (o, ln) in enumerate(cch):
                    hp = pg.tile([128, 512], DT.float32, tag="pg", name="hp")
                    for p in range(NPH):
                        nc.tensor.matmul(
                            hp[:, 0:ln],
                            w1ps[p][b][:, :, bo:bo + 128],
                            zps[p][:, :, o:o + ln],
                            start=(p == 0), stop=(p == NPH - 1),
                            perf_mode=DR,
                        )
                    nc.scalar.activation(hs[:, o:o + ln], hp[:, 0:ln], AF.Silu,
                                         scale=1.0 / SW1)
                    gp = pg.tile([128, 512], DT.float32, tag="pg", name="gp")
                    for p in range(NPH):
                        nc.tensor.matmul(
                            gp[:, 0:ln],
                            w3ps[p][b][:, :, bo:bo + 128],
                            zps[p][:, :, o:o + ln],
                            start=(p == 0), stop=(p == NPH - 1),
                            perf_mode=DR,
                        )
                    nc.vector.tensor_tensor(
                        hhp[ic // 2][:, ic % 2, o:o + ln],
                        gp[:, 0:ln], hs[:, o:o + ln], ALU.mult)
                if ic == 0:
                    # emit w2 load after the first h-block for DMA priority
                    w2ps = [wpool.tile([128, 2, H], DT.float8e4, name=f"w2p{p}",
                                       tag=f"w2p{p}") for p in range(NPI)]
                    w2r = w2T.rearrange("(a two p) m -> a p two m", p=128, two=2)
                    for p in range(NPI):
                        nc.sync.dma_start(w2ps[p][:], w2r[p])
                    w2_holder.append(w2ps)

            w2ps = w2_holder[0]
            for hc in range(NCI):
                yt = ytpool.tile([128, cap], DT.bfloat16, tag="yt", name="yt")
                for j, (o, ln) in enumerate(cch):
                    yp = py.tile([128, 512], DT.float32, tag="py", name="yp")
                    for p in range(NPI):
                        nc.tensor.matmul(
                            yp[:, 0:ln],
                            w2ps[p][:, :, hc * 128:(hc + 1) * 128],
                            hhp[p][:, :, o:o + ln],
                            start=(p == 0), stop=(p == NPI - 1),
                            perf_mode=DR,
                        )
                    nc.vector.tensor_tensor(
                        yt[:, o:o + ln], yp[:, 0:ln], web_sb[:, o:o + ln], ALU.mult)
                nc.sync.dma_start(yT[hc * 128:(hc + 1) * 128, :], yt[:])

    nc.compile()
    nc.finalize()
    return nc


def _get(name, builder, *args):
    if name not in _CACHE:
        _CACHE[name] = builder(*args)
    return _CACHE[name]


def _rmsnorm(x, w):
    xf = x.astype(np.float32)
    rms = 1.0 / np.sqrt((xf * xf).mean(axis=-1, keepdims=True) + EPS)
    return (xf * rms) * w.astype(np.float32)


def kernel(x, ln1_w, ln2_w, wq, wk, wv, wo, gate_w, w1, w2, w3):
    global LAST_RESULTS
    LAST_RESULTS = []
    x = np.asarray(x, np.float32)
    wq, wk, wv, wo = (np.asarray(a, np.float32) for a in (wq, wk, wv, wo))
    gate_w = np.asarray(gate_w, np.float32)
    w1, w2, w3 = (np.asarray(a, np.float32) for a in (w1, w2, w3))
    ln1_w = np.asarray(ln1_w, np.float32)
    ln2_w = np.asarray(ln2_w, np.float32)

    xf = x.reshape(T, H)
    z1 = _rmsnorm(xf, ln1_w)
    # ---- launch 1: attention (fp8 projections, bf16 scores/O-proj) ----
    SQK = 32.0
    nc1 = _get("l1", _build_l1)
    z1_8 = np.clip(z1, -240, 240).astype(F8)
    in_maps = []
    for c in range(NCORES):
        b, g = divmod(c, NGRP)
        sl = slice(g * DS, (g + 1) * DS)
        wqkv = np.concatenate([wq[sl].T, wk[sl].T, wv[sl].T], axis=1) * SQK
        in_maps.append({
            "xT8": np.ascontiguousarray(z1_8[b * S:(b + 1) * S].T),
            "wqkv8": np.clip(np.ascontiguousarray(wqkv), -240, 240).astype(F8),
            "woT": (np.ascontiguousarray(wo[:, sl].T) / SQK).astype(BF16),
        })
    res1 = run_bass_kernel_spmd(nc1, in_maps, core_ids=list(range(NCORES)), trace=TRACE)
    LAST_RESULTS.append(res1)

    h1 = xf.copy()
    for c in range(NCORES):
        b = c // NGRP
        h1[b * S:(b + 1) * S] += res1.results[c]["h1p"].astype(np.float32)

    # ---- host: routing (exact fp32 semantics like the reference) ----
    z = _rmsnorm(h1, ln2_w)
    logits = (z.astype(np.float64) @ gate_w.T.astype(np.float64)).astype(np.float32)
    order = np.argsort(-logits, axis=-1, kind="stable")
    sel = order[:, :2]                               # top-2, ties -> lower index
    vals = np.take_along_axis(logits, sel, axis=-1).astype(np.float32)
    mx = vals.max(axis=-1, keepdims=True)
    ex = np.exp(vals - mx)
    rw = (ex / ex.sum(axis=-1, keepdims=True)).astype(np.float32)

    idx_lists = []
    for e in range(E):
        m = (sel == e)
        tok = np.nonzero(m.any(axis=-1))[0]
        wgt = np.where(m, rw, 0.0).sum(axis=-1)[tok]
        idx_lists.append((tok, wgt.astype(np.float32)))
    maxload = max(len(tok) for tok, _ in idx_lists)
    cap = C
    while cap < maxload:
        cap += 512
    nc2 = _get(f"l2_{cap}", _build_l2, cap)

    # ---- launch 2: expert-parallel FFN (fp8) ----
    zT = np.clip(np.ascontiguousarray(z.T), -240, 240).astype(F8)    # [H, T]
    in_maps2 = []
    for e in range(E):
        tok, wgt = idx_lists[e]
        zeT = np.zeros((H, cap), F8)
        zeT[:, :len(tok)] = zT[:, tok]
        web = np.zeros((cap,), np.float32)
        web[:len(tok)] = wgt / (SW3 * SW2)
        in_maps2.append({
            "zeT": zeT,
            "w1T": np.clip(np.ascontiguousarray(w1[e].T) * SW1, -240, 240).astype(F8),
            "w3T": np.clip(np.ascontiguousarray(w3[e].T) * SW3, -240, 240).astype(F8),
            "w2T": np.clip(np.ascontiguousarray(w2[e].T) * SW2, -240, 240).astype(F8),
            "web": np.broadcast_to(web, (128, cap)).copy(),
        })
    res2 = run_bass_kernel_spmd(nc2, in_maps2, core_ids=list(range(NCORES)), trace=TRACE)
    LAST_RESULTS.append(res2)

    out = h1.copy()
    for e in range(E):
        tok, _ = idx_lists[e]
        out[tok] += res2.results[e]["yT"][:, :len(tok)].T.astype(np.float32)

    return out.reshape(B, S, H).astype(np.float32)

